# revision 39
# baseline (speedup 1.0000x reference)
"""Trainium2 Bass kernel for nn_DomainDiscriminator.

Network: conv(512->256,k3,s3,p1) -> BN -> conv(256->128,k3,s3,p1) -> BN
         -> reshape -> 12-layer MLP (3200->...->1, no nonlinearities) -> sigmoid.
Input x: [64, 512, 40, 40] f32.  Output: [64, 1] f32.

Strategy (8 NeuronCores, pure data parallel, 8 batch per core):
 - stride==kernel==3 convs are non-overlapping patch matmuls. Conv1 patches
   are built host-side (space-to-depth, bf16); conv2 patches are read out of
   SBUF with strided access patterns.
 - conv1 streams x in 18 small (k, half) tiles; the matmul loop is k-major
   within each half (pair of psum tiles per mt) so each weight tile is loaded
   once per 2 matmuls and the first matmul starts as soon as the first 392KB
   tile lands.
 - Training-mode BN: conv biases are absorbed exactly by BN; per-channel batch
   stats are computed per-psum-tile during conv1 (on DVE/ACT, which idle),
   exchanged with tiny 2KB AllGathers, tree-summed on chip.
 - ACT tables (Square/Sqrt/Sigmoid) are preloaded off the critical path;
   the BN2 AllGather is triggered before the weff partial matvec.
 - The 12 linear layers compose on the host (fp64) into a single [3200]
   vector + scalar bias; the device finishes with two tiny matmuls + sigmoid.
"""

import os
import sys

sys.path.insert(0, "/opt/trn_rl_repo")

import numpy as np

import concourse.bass as bass
import concourse.mybir as mybir
import concourse.tile as tile
from concourse import bacc
from concourse.bass_utils import run_bass_kernel_spmd

F32 = mybir.dt.float32
BF16 = mybir.dt.bfloat16

NCORES = 8
BL = 8              # batch per core
B = 64              # full batch
EPS = 1e-5

P1 = 196            # 14*14 conv1 positions
P2 = 25             # 5*5 conv2 positions
PTW = 2 * P1        # 392 cols per conv1 psum tile (2 batch)
N1 = float(B * P1)  # BN1 stat count
N2 = float(B * P2)  # BN2 stat count

_CACHE = {}

KIJ9 = [(ki, kj) for ki in range(3) for kj in range(3)]
# conv2 im2col: per (ki,kj) a full zero-padded 5*5*8 block in h1sb
BLK = 200
H1W = 9 * BLK       # 1800 cols per h1 tile


# ----------------------------------------------------------------------------
# device program
# ----------------------------------------------------------------------------

def _emit_casts(nc, work, h1sb):
    """psum -> h1sb bf16 im2col-block copies for one conv1 half."""
    half, ps = work
    for ptin in range(2):
        pt = half * 2 + ptin
        for mt in range(2):
            pr = ps[(ptin, mt)][:].rearrange("p (n i j) -> p n i j",
                                             n=2, i=14, j=14)
            for kidx, (ki, kj) in enumerate(KIJ9):
                ilo, icnt = (1, 4) if ki == 0 else (0, 5)
                jlo, jcnt = (1, 4) if kj == 0 else (0, 5)
                srcv = pr[:, :, 3 * ilo + ki - 1:14:3,
                          3 * jlo + kj - 1:14:3].transpose([0, 2, 3, 1])
                off = kidx * BLK + (ilo * 5 + jlo) * 8
                dstv = bass.AP(
                    h1sb[mt].tensor,
                    h1sb[mt].offset + off + 2 * pt,
                    [list(h1sb[mt].ap[0]), [40, icnt], [8, jcnt], [1, 2]])
                nc.vector.tensor_copy(dstv, srcv)


def _build():
    nc = bacc.Bacc("TRN2", target_bir_lowering=False, debug=False,
                   enable_asserts=True, num_devices=NCORES)

    # xprep: [9 kij, 2 half, 128, 4cb * 2ptin * 392]  (cb-major, ptin, cols)
    xprep = nc.dram_tensor("xprep", [9, 2, 128, 4 * 2 * PTW], BF16,
                           kind="ExternalInput")
    # w1p: [128, 9 kij, 4 cb, 256 m]
    w1p = nc.dram_tensor("w1p", [128, 9, 4 * 256], BF16, kind="ExternalInput")
    w2p = nc.dram_tensor("w2p", [128, 18, 128], BF16, kind="ExternalInput")
    # conv2 boundary-class row sums: [(class4, cb2) blocks][c1w, c2]
    w2sp = nc.dram_tensor("w2sp", [128, 8 * 128], BF16, kind="ExternalInput")
    weffp = nc.dram_tensor("weffp", [128, 26], F32, kind="ExternalInput")
    bprep = nc.dram_tensor("bprep", [128, 7], F32, kind="ExternalInput")
    out = nc.dram_tensor("out", [BL, 1], F32, kind="ExternalOutput")

    # bprep columns: bn1_g (2), bn1_b (2), bn2_g, bn2_b, beff(row 0)
    BC_BN1G, BC_BN1B, BC_BN2G, BC_BN2B, BC_BEFF = 0, 2, 4, 5, 6

    ISQ1 = 1.0 / np.sqrt(N1)   # Square-accum scale so accum = sum(h^2)/N1
    ISQ2 = 1.0 / np.sqrt(N2)

    with tile.TileContext(nc) as tc:
        with tc.tile_pool(name="wp", bufs=1) as wp, \
             tc.tile_pool(name="xp", bufs=5) as xp, \
             tc.tile_pool(name="hp", bufs=1) as hp, \
             tc.tile_pool(name="sp", bufs=1) as sp, \
             tc.tile_pool(name="cps", bufs=8, space="PSUM") as cps, \
             tc.tile_pool(name="dram", bufs=1, space="DRAM") as dram:

            # ---------------- first loads ------------------------------
            # x tiles stream on the Sync HWDGE queue; all weights ride the
            # Scalar queue so the x stream never stalls behind them.
            w1sb = wp.tile([128, 9 * 1024], BF16)
            xt = {}
            for half in range(2):
                for k in range(9):
                    xt[(k, half)] = xp.tile([128, 8 * PTW], BF16,
                                            name=f"xt{k}_{half}", tag="xt")
            nc.sync.dma_start(xt[(0, 0)][:, 0:4 * PTW],
                              xprep.ap()[0, 0][:, 0:4 * PTW])
            w1r = w1p.ap().rearrange("p a b -> p (a b)")
            nc.sync.dma_start(w1sb[:, 0:1024], w1r[:, 0:1024])
            nc.sync.dma_start(xt[(0, 0)][:, 4 * PTW:],
                              xprep.ap()[0, 0][:, 4 * PTW:])

            # ncfw warm-up: a tiny AllGather nobody consumes; hides the
            # TOPSP cold-start under conv1
            warm_in = dram.tile([1, 4], F32)
            warm_out = dram.tile([NCORES, 1, 4], F32, addr_space="Shared")
            dummy = sp.tile([1, 8], F32)
            nc.gpsimd.memset(dummy[:], 0.0)
            epst = sp.tile([128, 1], F32)
            nc.gpsimd.memset(epst[:], EPS)
            nc.scalar.dma_start(warm_in[:], dummy[:, 0:4])
            nc.gpsimd.collective_compute(
                "AllGather", mybir.AluOpType.bypass,
                replica_groups=[list(range(NCORES))],
                ins=[warm_in.opt()], outs=[warm_out.opt()])
            # ACT Square table preload while ACT is idle
            nc.scalar.activation(dummy[:, 4:5], dummy[:, 5:6],
                                 mybir.ActivationFunctionType.Square)

            # remaining streamed loads, interleaved with compute demand order
            for k in range(1, 9):
                nc.sync.dma_start(xt[(k, 0)][:], xprep.ap()[k, 0])
                nc.sync.dma_start(w1sb[:, k * 1024:(k + 1) * 1024],
                                  w1r[:, k * 1024:(k + 1) * 1024])
            for k in range(9):
                nc.sync.dma_start(xt[(k, 1)][:], xprep.ap()[k, 1])
            w2sb = wp.tile([128, 18 * 128], BF16)
            nc.sync.dma_start(w2sb[:], w2p.ap().rearrange("p a b -> p (a b)"))
            w2s_sb = wp.tile([128, 8 * 128], BF16)
            nc.sync.dma_start(w2s_sb[:], w2sp.ap())
            weff = wp.tile([128, 26], F32)
            nc.sync.dma_start(weff[:], weffp.ap())
            bsb = wp.tile([128, 7], F32)
            nc.sync.dma_start(bsb[:], bprep.ap())

            # ---------------- conv1 (k-major per half) ------------------
            scratch = sp.tile([128, PTW], F32)
            # h1 tiles hold RAW conv1 output in zero-padded 5*5*8 blocks
            # per (ki,kj); border slots stay zero (memset once)
            h1sb = [hp.tile([128, H1W], BF16, name=f"h1_{mt}")
                    for mt in range(2)]
            for mt in range(2):
                nc.gpsimd.memset(h1sb[mt][:], 0.0)
            # per-psum-tile stats: [128, 4pt * (2kind*2mt)]; kind0=sum, 1=sumsq
            stt = sp.tile([128, 16], F32)

            cast_work = []
            for half in range(2):
                ps = {}
                for ptin in range(2):
                    for mt in range(2):
                        ps[(ptin, mt)] = cps.tile([128, PTW], F32,
                                                  name=f"c1ps{half}{ptin}{mt}",
                                                  tag="c1ps")
                for k in range(9):
                    xk = xt[(k, half)][:].rearrange(
                        "p (c t w) -> p c t w", c=4, t=2)
                    for cb in range(4):
                        for mt in range(2):
                            lhsT = w1sb[:, (k * 4 + cb) * 256 + mt * 128:
                                        (k * 4 + cb) * 256 + (mt + 1) * 128]
                            for ptin in range(2):
                                nc.tensor.matmul(
                                    ps[(ptin, mt)][:], lhsT, xk[:, cb, ptin],
                                    start=(k == 0 and cb == 0),
                                    stop=(k == 8 and cb == 3))
                # stats first — they gate the AllGather trigger; the casts
                # drain later (half0's under half1's matmuls, half1's under
                # the collective). Sums on DVE, sum(h^2)/N1 on ACT.
                for ptin in range(2):
                    pt = half * 2 + ptin
                    for mt in range(2):
                        p = ps[(ptin, mt)]
                        nc.vector.reduce_sum(stt[:, pt * 4 + mt:pt * 4 + mt + 1],
                                             p[:], axis=mybir.AxisListType.X)
                        nc.scalar.activation(scratch[:], p[:],
                                             mybir.ActivationFunctionType.Square,
                                             scale=ISQ1,
                                             accum_out=stt[:, pt * 4 + 2 + mt:
                                                           pt * 4 + 3 + mt])
                cast_work.append((half, ps))
                if half == 0:
                    _emit_casts(nc, cast_work.pop(), h1sb)

            # ---------------- BN1 stats combine + AllGather -------------
            # combine 4 pt blocks: [128, 4pt, 4] -> [128, 4]
            nc.vector.tensor_tensor(stt[:, 0:8], stt[:, 0:8], stt[:, 8:16],
                                    op=mybir.AluOpType.add)
            nc.vector.tensor_tensor(stt[:, 0:4], stt[:, 0:4], stt[:, 4:8],
                                    op=mybir.AluOpType.add)
            st1 = stt[:, 0:4]    # [S_mt0, S_mt1, Q_mt0, Q_mt1]; Q pre-div N1
            bn1_in = dram.tile([128, 4], F32)
            bn1_out = dram.tile([NCORES, 128, 4], F32, addr_space="Shared")
            nc.scalar.dma_start(bn1_in[:], st1)
            nc.gpsimd.collective_compute(
                "AllGather", mybir.AluOpType.bypass,
                replica_groups=[list(range(NCORES))],
                ins=[bn1_in.opt()], outs=[bn1_out.opt()])
            # Sqrt table preload during the AllGather wait; the stt read
            # keeps it ordered after the stats Squares, scale=0 + eps bias
            # keeps the argument in sqrt's valid range
            nc.scalar.activation(dummy[:, 4:5], stt[0:1, 0:1],
                                 mybir.ActivationFunctionType.Sqrt,
                                 scale=0.0, bias=epst[0:1, 0:1])
            # half1's im2col casts drain during the AllGather
            _emit_casts(nc, cast_work.pop(), h1sb)
            stg = sp.tile([128, NCORES * 4], F32)
            nc.scalar.dma_start(
                stg[:].rearrange("p (r t) -> p r t", r=NCORES),
                bass.AP(bn1_out.tensor, 0, [[4, 128], [128 * 4, NCORES], [1, 4]]))
            stgr = stg[:].rearrange("p (r t) -> p r t", r=NCORES)
            for halfn in (4, 2, 1):
                nc.vector.tensor_tensor(
                    stgr[:, 0:halfn], stgr[:, 0:halfn],
                    stgr[:, halfn:2 * halfn], op=mybir.AluOpType.add)

            # BN1 coeffs, scale-first so conv2's weight scaling can start
            # before the shift path finishes. BN1 is folded into conv2:
            # the per-input-channel scale goes into w2 (h1sb's zero padding
            # must stay zero), the shift's contribution is a per-boundary-
            # class constant from 8 tiny matmuls against host-precomputed
            # class row sums.
            t1 = sp.tile([128, 8], F32, name="bn1t")
            mean1, var1, sd1, tn1 = (t1[:, i * 2:(i + 1) * 2] for i in range(4))
            nc.vector.tensor_scalar(mean1, stg[:, 0:2], 1.0 / N1, None,
                                    op0=mybir.AluOpType.mult)
            nc.vector.tensor_tensor(var1, mean1, mean1, op=mybir.AluOpType.mult)
            nc.vector.tensor_tensor(var1, stg[:, 2:4], var1,
                                    op=mybir.AluOpType.subtract)
            nc.scalar.activation(sd1, var1, mybir.ActivationFunctionType.Sqrt,
                                 bias=epst[:, 0:1])
            co1 = sp.tile([128, 6], F32, name="bn1c")
            scale1, shift1, r1 = co1[:, 0:2], co1[:, 2:4], co1[:, 4:6]
            nc.vector.reciprocal(r1, sd1)
            nc.vector.tensor_tensor(scale1, bsb[:, BC_BN1G:BC_BN1G + 2], r1,
                                    op=mybir.AluOpType.mult)
            for cb2 in range(2):
                nc.vector.tensor_scalar(
                    w2sb[:, cb2 * 1152:(cb2 + 1) * 1152],
                    w2sb[:, cb2 * 1152:(cb2 + 1) * 1152],
                    scale1[:, cb2:cb2 + 1], None, op0=mybir.AluOpType.mult)
            nc.vector.tensor_tensor(tn1, mean1, scale1, op=mybir.AluOpType.mult)
            nc.vector.tensor_tensor(shift1, bsb[:, BC_BN1B:BC_BN1B + 2], tn1,
                                    op=mybir.AluOpType.subtract)
            shift1b = sp.tile([128, 2], BF16)
            nc.vector.tensor_copy(shift1b[:], shift1)
            Tm = cps.tile([128, 4], F32, tag="c1ps")
            for cls in range(4):
                for cb2 in range(2):
                    nc.tensor.matmul(Tm[:, cls:cls + 1],
                                     w2s_sb[:, (cls * 2 + cb2) * 128:
                                            (cls * 2 + cb2 + 1) * 128],
                                     shift1b[:, cb2:cb2 + 1],
                                     start=(cb2 == 0), stop=(cb2 == 1),
                                     skip_group_check=True)
            TmS = sp.tile([128, 4], F32)
            nc.vector.tensor_copy(TmS[:], Tm[:])
            # broadcast the class constants into a full [128,200] map while
            # conv2 runs, so the post-conv2 merge is a single add
            Tmap = sp.tile([128, BL * P2], F32)
            Tv = Tmap[:].rearrange("p (i j n) -> p i j n", i=5, j=5, n=BL)
            for cls, sl in ((3, (slice(0, 1), slice(0, 1))),
                            (2, (slice(0, 1), slice(1, 5))),
                            (1, (slice(1, 5), slice(0, 1))),
                            (0, (slice(1, 5), slice(1, 5)))):
                ni = sl[0].stop - sl[0].start
                nj = (sl[1].stop - sl[1].start) * BL
                src = TmS[:, cls:cls + 1, None].to_broadcast([128, ni, nj])
                nc.vector.tensor_copy(
                    Tv[:, sl[0], sl[1]].rearrange("p a b c -> p a (b c)"), src)

            # ---------------- conv2 (one 18-matmul chain) ---------------
            c2p = cps.tile([128, P2 * BL], F32, name="c2p", tag="c1ps")
            for idx in range(18):
                cb2, kidx = idx // 9, idx % 9
                lhsT = w2sb[:, idx * 128:(idx + 1) * 128]
                nc.tensor.matmul(c2p[:], lhsT,
                                 h1sb[cb2][:, kidx * BLK:(kidx + 1) * BLK],
                                 start=(idx == 0), stop=(idx == 17))
            # psum -> sbuf bf16, adding the BN1-shift map in one pass
            c2sb = sp.tile([128, BL * P2], BF16)
            nc.vector.tensor_tensor(c2sb[:], Tmap[:], c2p[:],
                                    op=mybir.AluOpType.add)

            # ---------------- BN2 stats + AllGather (before matvec) -----
            st2l = sp.tile([128, 2], F32)
            nc.vector.reduce_sum(st2l[:, 0:1], c2sb[:], axis=mybir.AxisListType.X)
            sc2 = sp.tile([128, BL * P2], F32)
            nc.scalar.activation(sc2[:], c2sb[:],
                                 mybir.ActivationFunctionType.Square,
                                 scale=ISQ2, accum_out=st2l[:, 1:2])
            bn2_in = dram.tile([128, 2], F32)
            bn2_out = dram.tile([NCORES, 128, 2], F32, addr_space="Shared")
            nc.scalar.dma_start(bn2_in[:], st2l[:])
            nc.gpsimd.collective_compute(
                "AllGather", mybir.AluOpType.bypass,
                replica_groups=[list(range(NCORES))],
                ins=[bn2_in.opt()], outs=[bn2_out.opt()])

            # during the AllGather: weff partial matvec + Sigmoid table +
            # the pieces of the finish that don't need stats
            mvt = sp.tile([128, P2 * BL], F32)
            wb = weff[:, 0:25, None].to_broadcast([128, 25, BL])
            nc.vector.tensor_tensor(
                mvt[:].rearrange("p (i n) -> p i n", i=P2),
                c2sb[:].rearrange("p (i n) -> p i n", i=P2), wb,
                op=mybir.AluOpType.mult)
            Av = sp.tile([128, BL], F32)
            nc.vector.reduce_sum(Av[:], mvt[:].rearrange("p (i n) -> p n i", i=P2),
                                 axis=mybir.AxisListType.X)
            Avb = sp.tile([128, BL], BF16)
            nc.vector.tensor_copy(Avb[:], Av[:])
            nc.scalar.activation(dummy[:, 4:5], st2l[0:1, 0:1],
                                 mybir.ActivationFunctionType.Sigmoid)
            ones = wp.tile([128, BL], BF16)
            nc.gpsimd.memset(ones[:], 1.0)

            stg2 = sp.tile([128, NCORES * 2], F32)
            nc.scalar.dma_start(
                stg2[:].rearrange("p (r t) -> p r t", r=NCORES),
                bass.AP(bn2_out.tensor, 0, [[2, 128], [128 * 2, NCORES], [1, 2]]))
            stg2r = stg2[:].rearrange("p (r t) -> p r t", r=NCORES)
            for halfn in (4, 2, 1):
                nc.vector.tensor_tensor(
                    stg2r[:, 0:halfn], stg2r[:, 0:halfn],
                    stg2r[:, halfn:2 * halfn], op=mybir.AluOpType.add)

            # ---------------- BN2 coeffs + collapsed MLP finish ---------
            # z[n] = sum_c s2[c]*A[c,n] + sum_c shift2[c]*rowsum_weff[c];
            # scale-first so the first matmul issues while the shift path
            # is still on DVE
            t2 = sp.tile([128, 4], F32, name="bn2t")
            mean2, var2, sd2, tn2 = (t2[:, i:i + 1] for i in range(4))
            nc.vector.tensor_scalar(mean2, stg2[:, 0:1], 1.0 / N2, None,
                                    op0=mybir.AluOpType.mult)
            nc.vector.tensor_tensor(var2, mean2, mean2, op=mybir.AluOpType.mult)
            nc.vector.tensor_tensor(var2, stg2[:, 1:2], var2,
                                    op=mybir.AluOpType.subtract)
            nc.scalar.activation(sd2, var2, mybir.ActivationFunctionType.Sqrt,
                                 bias=epst[:, 0:1])
            co2 = sp.tile([128, 3], F32, name="bn2c")
            scale2, shift2, r2 = co2[:, 0:1], co2[:, 1:2], co2[:, 2:3]
            nc.vector.reciprocal(r2, sd2)
            nc.vector.tensor_tensor(scale2, bsb[:, BC_BN2G:BC_BN2G + 1], r2,
                                    op=mybir.AluOpType.mult)
            s2b = sp.tile([128, 1], BF16)
            nc.vector.tensor_copy(s2b[:], scale2)
            zps = cps.tile([1, BL], F32, tag="c1ps")
            nc.tensor.matmul(zps[:], s2b[:], Avb[:], start=True, stop=False)
            nc.vector.tensor_tensor(tn2, mean2, scale2, op=mybir.AluOpType.mult)
            nc.vector.tensor_tensor(shift2, bsb[:, BC_BN2B:BC_BN2B + 1], tn2,
                                    op=mybir.AluOpType.subtract)
            vsh = wp.tile([128, 1], BF16)
            nc.vector.tensor_tensor(vsh[:], shift2, weff[:, 25:26],
                                    op=mybir.AluOpType.mult)
            nc.tensor.matmul(zps[:], vsh[:], ones[:], start=False, stop=True)
            osb = sp.tile([1, BL], F32)
            nc.scalar.activation(osb[:], zps[:],
                                 mybir.ActivationFunctionType.Sigmoid,
                                 bias=bsb[0:1, BC_BEFF:BC_BEFF + 1])
            nc.sync.dma_start(bass.AP(out, 0, [[1, 1], [1, BL]]), osb[:])

    nc.compile()
    return nc


# ----------------------------------------------------------------------------
# host-side input prep
# ----------------------------------------------------------------------------

def _prep_inputs(inputs):
    import ml_dtypes
    f = np.float32
    bf = ml_dtypes.bfloat16
    x = np.asarray(inputs["x"], dtype=f)

    # conv1 patches: [n64, cb4, c128, i14, ki3, j14, kj3]
    xpad = np.zeros((B, 512, 42, 42), dtype=bf)
    xpad[:, :, 1:41, 1:41] = x.astype(bf)
    # -> [k9, cb4, c128, n64, pos196]
    xv = (xpad.reshape(B, 4, 128, 14, 3, 14, 3)
          .transpose(4, 6, 1, 2, 0, 3, 5)        # ki,kj,cb,c,n,i,j
          .reshape(9, 4, 128, B, P1))

    w1 = np.asarray(inputs["conv1_w"], dtype=f)          # [256, 512, 3, 3]
    # [128c, 9k, 4cb, 256m]
    w1p = np.ascontiguousarray(
        w1.reshape(256, 4, 128, 9).transpose(2, 3, 1, 0)).reshape(
            128, 9, 1024).astype(bf)
    w2 = np.asarray(inputs["conv2_w"], dtype=f)          # [128, 256, 3, 3]
    w2p = np.ascontiguousarray(
        w2.reshape(128, 2, 128, 9).transpose(2, 1, 3, 0)).reshape(
            128, 18, 128).astype(bf)
    # boundary-class row sums for the BN1-shift term:
    # class c = a*2+b, a=(i==0) -> ki>=1 only, b=(j==0) -> kj>=1 only
    w2r = w2.reshape(128, 2, 128, 3, 3)                  # c2, cb, c1w, ki, kj
    w2sp = np.zeros((128, 8 * 128), dtype=f)
    for cls in range(4):
        a, b = cls // 2, cls % 2
        kis = slice(1, 3) if a else slice(0, 3)
        kjs = slice(1, 3) if b else slice(0, 3)
        s = w2r[:, :, :, kis, kjs].sum(axis=(3, 4))      # c2, cb, c1w
        for cb in range(2):
            w2sp[:, (cls * 2 + cb) * 128:(cls * 2 + cb + 1) * 128] = s[:, cb].T
    w2sp = w2sp.astype(bf)

    # compose the 12 affine layers (no nonlinearities) into [3200] + scalar
    M = np.asarray(inputs["w14"], dtype=np.float64)      # [1, 2]
    beff = np.asarray(inputs["b14"], dtype=np.float64).copy()  # [1]
    for li in range(13, 2, -1):                          # w13 .. w3
        beff += M @ np.asarray(inputs[f"b{li}"], dtype=np.float64)
        M = M @ np.asarray(inputs[f"w{li}"], dtype=np.float64)
    weff = M.reshape(3200).astype(f)                     # order f = c*25 + ij
    w2d = weff.reshape(128, 25)
    weffp = np.zeros((128, 26), dtype=f)
    weffp[:, 0:25] = w2d
    weffp[:, 25] = w2d.sum(axis=1)
    beff_f = float(beff[0])

    bp = np.zeros((128, 7), dtype=f)
    bp[:, 0:2] = np.asarray(inputs["bn1_g"], dtype=f).reshape(2, 128).T
    bp[:, 2:4] = np.asarray(inputs["bn1_b"], dtype=f).reshape(2, 128).T
    bp[:, 4] = np.asarray(inputs["bn2_g"], dtype=f)
    bp[:, 5] = np.asarray(inputs["bn2_b"], dtype=f)
    bp[0, 6] = beff_f

    in_maps = []
    for r in range(NCORES):
        # [9k, 4cb, 128, 8n, 196] -> [9k, 2half, 128, 4cb, 2ptin, 2n, 196]
        xr = np.ascontiguousarray(
            xv[:, :, :, r * BL:(r + 1) * BL]
            .reshape(9, 4, 128, 2, 2, 2, P1)     # k, cb, c, half, ptin, n2, pos
            .transpose(0, 3, 2, 1, 4, 5, 6)
        ).reshape(9, 2, 128, 4 * 2 * PTW)
        in_maps.append({
            "xprep": xr, "w1p": w1p, "w2p": w2p, "w2sp": w2sp,
            "weffp": weffp, "bprep": bp,
        })
    return in_maps


def kernel(**inputs):
    if "nc" not in _CACHE:
        _CACHE["nc"] = _build()
    nc = _CACHE["nc"]
    in_maps = _prep_inputs(inputs)
    trace = bool(int(os.environ.get("KERNEL_TRACE", "0")))
    if trace:
        try:
            import ntff_shim
            ntff_shim.install()
        except ImportError:
            trace = False
    res = run_bass_kernel_spmd(nc, in_maps, core_ids=list(range(NCORES)),
                               trace=trace)
    _CACHE["last_result"] = res
    return np.concatenate([res.results[r]["out"] for r in range(NCORES)], axis=0)


# revision 40
# speedup vs baseline: 1.0373x; 1.0373x over previous
"""Trainium2 Bass kernel for nn_DomainDiscriminator.

Network: conv(512->256,k3,s3,p1) -> BN -> conv(256->128,k3,s3,p1) -> BN
         -> reshape -> 12-layer MLP (3200->...->1, no nonlinearities) -> sigmoid.
Input x: [64, 512, 40, 40] f32.  Output: [64, 1] f32.

Strategy (8 NeuronCores, pure data parallel, 8 batch per core):
 - stride==kernel==3 convs are non-overlapping patch matmuls. Conv1 patches
   are built host-side (space-to-depth, bf16); conv2 patches are read out of
   SBUF with strided access patterns.
 - conv1 streams x in 18 small (k, half) tiles; the matmul loop is k-major
   within each half (pair of psum tiles per mt) so each weight tile is loaded
   once per 2 matmuls and the first matmul starts as soon as the first 392KB
   tile lands.
 - Training-mode BN: conv biases are absorbed exactly by BN; per-channel batch
   stats are computed per-psum-tile during conv1 (on DVE/ACT, which idle),
   exchanged with tiny 2KB AllGathers, tree-summed on chip.
 - ACT tables (Square/Sqrt/Sigmoid) are preloaded off the critical path;
   the BN2 AllGather is triggered before the weff partial matvec.
 - The 12 linear layers compose on the host (fp64) into a single [3200]
   vector + scalar bias; the device finishes with two tiny matmuls + sigmoid.
"""

import os
import sys

sys.path.insert(0, "/opt/trn_rl_repo")

import numpy as np

import concourse.bass as bass
import concourse.mybir as mybir
import concourse.tile as tile
from concourse import bacc
from concourse.bass_utils import run_bass_kernel_spmd

F32 = mybir.dt.float32
BF16 = mybir.dt.bfloat16

NCORES = 8
BL = 8              # batch per core
B = 64              # full batch
EPS = 1e-5

P1 = 196            # 14*14 conv1 positions
P2 = 25             # 5*5 conv2 positions
PTW = 2 * P1        # 392 cols per conv1 psum tile (2 batch)
N1 = float(B * P1)  # BN1 stat count
N2 = float(B * P2)  # BN2 stat count

_CACHE = {}

KIJ9 = [(ki, kj) for ki in range(3) for kj in range(3)]
# conv2 im2col: per (ki,kj) a full zero-padded 5*5*8 block in h1sb
BLK = 200
H1W = 9 * BLK       # 1800 cols per h1 tile


# ----------------------------------------------------------------------------
# device program
# ----------------------------------------------------------------------------

def _emit_casts(nc, work, h1sb):
    """psum -> h1sb bf16 im2col-block copies for one conv1 half."""
    half, ps = work
    for ptin in range(2):
        pt = half * 2 + ptin
        for mt in range(2):
            pr = ps[(ptin, mt)][:].rearrange("p (n i j) -> p n i j",
                                             n=2, i=14, j=14)
            for kidx, (ki, kj) in enumerate(KIJ9):
                ilo, icnt = (1, 4) if ki == 0 else (0, 5)
                jlo, jcnt = (1, 4) if kj == 0 else (0, 5)
                srcv = pr[:, :, 3 * ilo + ki - 1:14:3,
                          3 * jlo + kj - 1:14:3].transpose([0, 2, 3, 1])
                off = kidx * BLK + (ilo * 5 + jlo) * 8
                dstv = bass.AP(
                    h1sb[mt].tensor,
                    h1sb[mt].offset + off + 2 * pt,
                    [list(h1sb[mt].ap[0]), [40, icnt], [8, jcnt], [1, 2]])
                nc.vector.tensor_copy(dstv, srcv)


def _build():
    nc = bacc.Bacc("TRN2", target_bir_lowering=False, debug=False,
                   enable_asserts=True, num_devices=NCORES)

    # xprep: [9 kij, 2 half, 128, 4cb * 2ptin * 392]  (cb-major, ptin, cols)
    xprep = nc.dram_tensor("xprep", [9, 2, 128, 4 * 2 * PTW], BF16,
                           kind="ExternalInput")
    # w1p: [128, 9 kij, 4 cb, 256 m]
    w1p = nc.dram_tensor("w1p", [128, 9, 4 * 256], BF16, kind="ExternalInput")
    w2p = nc.dram_tensor("w2p", [128, 18, 128], BF16, kind="ExternalInput")
    # conv2 boundary-class row sums: [(class4, cb2) blocks][c1w, c2]
    w2sp = nc.dram_tensor("w2sp", [128, 8 * 128], BF16, kind="ExternalInput")
    weffp = nc.dram_tensor("weffp", [128, 26], F32, kind="ExternalInput")
    bprep = nc.dram_tensor("bprep", [128, 7], F32, kind="ExternalInput")
    out = nc.dram_tensor("out", [BL, 1], F32, kind="ExternalOutput")

    # bprep columns: bn1_g (2), bn1_b (2), bn2_g, bn2_b, beff(row 0)
    BC_BN1G, BC_BN1B, BC_BN2G, BC_BN2B, BC_BEFF = 0, 2, 4, 5, 6

    ISQ1 = 1.0 / np.sqrt(N1)   # Square-accum scale so accum = sum(h^2)/N1
    ISQ2 = 1.0 / np.sqrt(N2)

    with tile.TileContext(nc) as tc:
        with tc.tile_pool(name="wp", bufs=1) as wp, \
             tc.tile_pool(name="xp", bufs=7) as xp, \
             tc.tile_pool(name="hp", bufs=1) as hp, \
             tc.tile_pool(name="sp", bufs=1) as sp, \
             tc.tile_pool(name="cps", bufs=8, space="PSUM") as cps, \
             tc.tile_pool(name="dram", bufs=1, space="DRAM") as dram:

            # ---------------- first loads ------------------------------
            # x tiles stream on the Sync HWDGE queue; all weights ride the
            # Scalar queue so the x stream never stalls behind them.
            w1sb = wp.tile([128, 9 * 1024], BF16)
            xt = {}
            for half in range(2):
                for k in range(9):
                    xt[(k, half)] = xp.tile([128, 8 * PTW], BF16,
                                            name=f"xt{k}_{half}", tag="xt")
            nc.sync.dma_start(xt[(0, 0)][:, 0:4 * PTW],
                              xprep.ap()[0, 0][:, 0:4 * PTW])
            w1r = w1p.ap().rearrange("p a b -> p (a b)")
            nc.sync.dma_start(w1sb[:, 0:1024], w1r[:, 0:1024])
            nc.sync.dma_start(xt[(0, 0)][:, 4 * PTW:],
                              xprep.ap()[0, 0][:, 4 * PTW:])

            # ncfw warm-up: a tiny AllGather nobody consumes; hides the
            # TOPSP cold-start under conv1
            warm_in = dram.tile([1, 4], F32)
            warm_out = dram.tile([NCORES, 1, 4], F32, addr_space="Shared")
            dummy = sp.tile([1, 8], F32)
            nc.gpsimd.memset(dummy[:], 0.0)
            epst = sp.tile([128, 1], F32)
            nc.gpsimd.memset(epst[:], EPS)
            nc.scalar.dma_start(warm_in[:], dummy[:, 0:4])
            nc.gpsimd.collective_compute(
                "AllGather", mybir.AluOpType.bypass,
                replica_groups=[list(range(NCORES))],
                ins=[warm_in.opt()], outs=[warm_out.opt()])
            # ACT Square table preload while ACT is idle
            nc.scalar.activation(dummy[:, 4:5], dummy[:, 5:6],
                                 mybir.ActivationFunctionType.Square)

            # remaining streamed loads, interleaved with compute demand order
            for k in range(1, 9):
                nc.sync.dma_start(xt[(k, 0)][:], xprep.ap()[k, 0])
                nc.sync.dma_start(w1sb[:, k * 1024:(k + 1) * 1024],
                                  w1r[:, k * 1024:(k + 1) * 1024])
            for k in range(9):
                nc.sync.dma_start(xt[(k, 1)][:], xprep.ap()[k, 1])
            w2sb = wp.tile([128, 18 * 128], BF16)
            nc.sync.dma_start(w2sb[:], w2p.ap().rearrange("p a b -> p (a b)"))
            w2s_sb = wp.tile([128, 8 * 128], BF16)
            nc.sync.dma_start(w2s_sb[:], w2sp.ap())
            weff = wp.tile([128, 26], F32)
            nc.sync.dma_start(weff[:], weffp.ap())
            bsb = wp.tile([128, 7], F32)
            nc.sync.dma_start(bsb[:], bprep.ap())

            # ---------------- conv1 (k-major per half) ------------------
            scratch = sp.tile([128, PTW], F32)
            # h1 tiles hold RAW conv1 output in zero-padded 5*5*8 blocks
            # per (ki,kj); border slots stay zero (memset once)
            h1sb = [hp.tile([128, H1W], BF16, name=f"h1_{mt}")
                    for mt in range(2)]
            for mt in range(2):
                nc.gpsimd.memset(h1sb[mt][:], 0.0)
            # per-psum-tile stats: [128, 4pt * (2kind*2mt)]; kind0=sum, 1=sumsq
            stt = sp.tile([128, 16], F32)

            cast_work = []
            for half in range(2):
                ps = {}
                for ptin in range(2):
                    for mt in range(2):
                        ps[(ptin, mt)] = cps.tile([128, PTW], F32,
                                                  name=f"c1ps{half}{ptin}{mt}",
                                                  tag="c1ps")
                for k in range(9):
                    xk = xt[(k, half)][:].rearrange(
                        "p (c t w) -> p c t w", c=4, t=2)
                    for cb in range(4):
                        for mt in range(2):
                            lhsT = w1sb[:, (k * 4 + cb) * 256 + mt * 128:
                                        (k * 4 + cb) * 256 + (mt + 1) * 128]
                            for ptin in range(2):
                                nc.tensor.matmul(
                                    ps[(ptin, mt)][:], lhsT, xk[:, cb, ptin],
                                    start=(k == 0 and cb == 0),
                                    stop=(k == 8 and cb == 3))
                # stats first — they gate the AllGather trigger; the casts
                # drain later (half0's under half1's matmuls, half1's under
                # the collective). Sums on DVE, sum(h^2)/N1 on ACT.
                for ptin in range(2):
                    pt = half * 2 + ptin
                    for mt in range(2):
                        p = ps[(ptin, mt)]
                        nc.vector.reduce_sum(stt[:, pt * 4 + mt:pt * 4 + mt + 1],
                                             p[:], axis=mybir.AxisListType.X)
                        nc.scalar.activation(scratch[:], p[:],
                                             mybir.ActivationFunctionType.Square,
                                             scale=ISQ1,
                                             accum_out=stt[:, pt * 4 + 2 + mt:
                                                           pt * 4 + 3 + mt])
                cast_work.append((half, ps))
                if half == 0:
                    _emit_casts(nc, cast_work.pop(), h1sb)

            # ---------------- BN1 stats combine + AllGather -------------
            # combine 4 pt blocks: [128, 4pt, 4] -> [128, 4]
            nc.vector.tensor_tensor(stt[:, 0:8], stt[:, 0:8], stt[:, 8:16],
                                    op=mybir.AluOpType.add)
            nc.vector.tensor_tensor(stt[:, 0:4], stt[:, 0:4], stt[:, 4:8],
                                    op=mybir.AluOpType.add)
            st1 = stt[:, 0:4]    # [S_mt0, S_mt1, Q_mt0, Q_mt1]; Q pre-div N1
            bn1_in = dram.tile([128, 4], F32)
            bn1_out = dram.tile([NCORES, 128, 4], F32, addr_space="Shared")
            nc.scalar.dma_start(bn1_in[:], st1)
            nc.gpsimd.collective_compute(
                "AllGather", mybir.AluOpType.bypass,
                replica_groups=[list(range(NCORES))],
                ins=[bn1_in.opt()], outs=[bn1_out.opt()])
            # Sqrt table preload during the AllGather wait; the stt read
            # keeps it ordered after the stats Squares, scale=0 + eps bias
            # keeps the argument in sqrt's valid range
            nc.scalar.activation(dummy[:, 4:5], stt[0:1, 0:1],
                                 mybir.ActivationFunctionType.Sqrt,
                                 scale=0.0, bias=epst[0:1, 0:1])
            # half1's im2col casts drain during the AllGather
            _emit_casts(nc, cast_work.pop(), h1sb)
            stg = sp.tile([128, NCORES * 4], F32)
            nc.scalar.dma_start(
                stg[:].rearrange("p (r t) -> p r t", r=NCORES),
                bass.AP(bn1_out.tensor, 0, [[4, 128], [128 * 4, NCORES], [1, 4]]))
            stgr = stg[:].rearrange("p (r t) -> p r t", r=NCORES)
            for halfn in (4, 2, 1):
                nc.vector.tensor_tensor(
                    stgr[:, 0:halfn], stgr[:, 0:halfn],
                    stgr[:, halfn:2 * halfn], op=mybir.AluOpType.add)

            # BN1 coeffs, scale-first so conv2's weight scaling can start
            # before the shift path finishes. BN1 is folded into conv2:
            # the per-input-channel scale goes into w2 (h1sb's zero padding
            # must stay zero), the shift's contribution is a per-boundary-
            # class constant from 8 tiny matmuls against host-precomputed
            # class row sums.
            t1 = sp.tile([128, 8], F32, name="bn1t")
            mean1, var1, sd1, tn1 = (t1[:, i * 2:(i + 1) * 2] for i in range(4))
            nc.vector.tensor_scalar(mean1, stg[:, 0:2], 1.0 / N1, None,
                                    op0=mybir.AluOpType.mult)
            nc.vector.tensor_tensor(var1, mean1, mean1, op=mybir.AluOpType.mult)
            nc.vector.tensor_tensor(var1, stg[:, 2:4], var1,
                                    op=mybir.AluOpType.subtract)
            nc.scalar.activation(sd1, var1, mybir.ActivationFunctionType.Sqrt,
                                 bias=epst[:, 0:1])
            co1 = sp.tile([128, 6], F32, name="bn1c")
            scale1, shift1, r1 = co1[:, 0:2], co1[:, 2:4], co1[:, 4:6]
            nc.vector.reciprocal(r1, sd1)
            nc.vector.tensor_tensor(scale1, bsb[:, BC_BN1G:BC_BN1G + 2], r1,
                                    op=mybir.AluOpType.mult)
            for cb2 in range(2):
                nc.vector.tensor_scalar(
                    w2sb[:, cb2 * 1152:(cb2 + 1) * 1152],
                    w2sb[:, cb2 * 1152:(cb2 + 1) * 1152],
                    scale1[:, cb2:cb2 + 1], None, op0=mybir.AluOpType.mult)
            nc.vector.tensor_tensor(tn1, mean1, scale1, op=mybir.AluOpType.mult)
            nc.vector.tensor_tensor(shift1, bsb[:, BC_BN1B:BC_BN1B + 2], tn1,
                                    op=mybir.AluOpType.subtract)
            shift1b = sp.tile([128, 2], BF16)
            nc.vector.tensor_copy(shift1b[:], shift1)
            Tm = cps.tile([128, 4], F32, tag="c1ps")
            for cls in range(4):
                for cb2 in range(2):
                    nc.tensor.matmul(Tm[:, cls:cls + 1],
                                     w2s_sb[:, (cls * 2 + cb2) * 128:
                                            (cls * 2 + cb2 + 1) * 128],
                                     shift1b[:, cb2:cb2 + 1],
                                     start=(cb2 == 0), stop=(cb2 == 1),
                                     skip_group_check=True)
            TmS = sp.tile([128, 4], F32)
            nc.vector.tensor_copy(TmS[:], Tm[:])
            # broadcast the class constants into a full [128,200] map while
            # conv2 runs, so the post-conv2 merge is a single add
            Tmap = sp.tile([128, BL * P2], F32)
            Tv = Tmap[:].rearrange("p (i j n) -> p i j n", i=5, j=5, n=BL)
            for cls, sl in ((3, (slice(0, 1), slice(0, 1))),
                            (2, (slice(0, 1), slice(1, 5))),
                            (1, (slice(1, 5), slice(0, 1))),
                            (0, (slice(1, 5), slice(1, 5)))):
                ni = sl[0].stop - sl[0].start
                nj = (sl[1].stop - sl[1].start) * BL
                src = TmS[:, cls:cls + 1, None].to_broadcast([128, ni, nj])
                nc.vector.tensor_copy(
                    Tv[:, sl[0], sl[1]].rearrange("p a b c -> p a (b c)"), src)

            # ---------------- conv2 (one 18-matmul chain) ---------------
            c2p = cps.tile([128, P2 * BL], F32, name="c2p", tag="c1ps")
            for idx in range(18):
                cb2, kidx = idx // 9, idx % 9
                lhsT = w2sb[:, idx * 128:(idx + 1) * 128]
                nc.tensor.matmul(c2p[:], lhsT,
                                 h1sb[cb2][:, kidx * BLK:(kidx + 1) * BLK],
                                 start=(idx == 0), stop=(idx == 17))
            # psum -> sbuf bf16, adding the BN1-shift map in one pass
            c2sb = sp.tile([128, BL * P2], BF16)
            nc.vector.tensor_tensor(c2sb[:], Tmap[:], c2p[:],
                                    op=mybir.AluOpType.add)

            # ---------------- BN2 stats + AllGather (before matvec) -----
            st2l = sp.tile([128, 2], F32)
            nc.vector.reduce_sum(st2l[:, 0:1], c2sb[:], axis=mybir.AxisListType.X)
            sc2 = sp.tile([128, BL * P2], F32)
            nc.scalar.activation(sc2[:], c2sb[:],
                                 mybir.ActivationFunctionType.Square,
                                 scale=ISQ2, accum_out=st2l[:, 1:2])
            bn2_in = dram.tile([128, 2], F32)
            bn2_out = dram.tile([NCORES, 128, 2], F32, addr_space="Shared")
            nc.scalar.dma_start(bn2_in[:], st2l[:])
            nc.gpsimd.collective_compute(
                "AllGather", mybir.AluOpType.bypass,
                replica_groups=[list(range(NCORES))],
                ins=[bn2_in.opt()], outs=[bn2_out.opt()])

            # during the AllGather: weff partial matvec + Sigmoid table +
            # the pieces of the finish that don't need stats
            mvt = sp.tile([128, P2 * BL], F32)
            wb = weff[:, 0:25, None].to_broadcast([128, 25, BL])
            nc.vector.tensor_tensor(
                mvt[:].rearrange("p (i n) -> p i n", i=P2),
                c2sb[:].rearrange("p (i n) -> p i n", i=P2), wb,
                op=mybir.AluOpType.mult)
            Av = sp.tile([128, BL], F32)
            nc.vector.reduce_sum(Av[:], mvt[:].rearrange("p (i n) -> p n i", i=P2),
                                 axis=mybir.AxisListType.X)
            Avb = sp.tile([128, BL], BF16)
            nc.vector.tensor_copy(Avb[:], Av[:])
            nc.scalar.activation(dummy[:, 4:5], st2l[0:1, 0:1],
                                 mybir.ActivationFunctionType.Sigmoid)
            ones = wp.tile([128, BL], BF16)
            nc.gpsimd.memset(ones[:], 1.0)

            stg2 = sp.tile([128, NCORES * 2], F32)
            nc.scalar.dma_start(
                stg2[:].rearrange("p (r t) -> p r t", r=NCORES),
                bass.AP(bn2_out.tensor, 0, [[2, 128], [128 * 2, NCORES], [1, 2]]))
            stg2r = stg2[:].rearrange("p (r t) -> p r t", r=NCORES)
            for halfn in (4, 2, 1):
                nc.vector.tensor_tensor(
                    stg2r[:, 0:halfn], stg2r[:, 0:halfn],
                    stg2r[:, halfn:2 * halfn], op=mybir.AluOpType.add)

            # ---------------- BN2 coeffs + collapsed MLP finish ---------
            # z[n] = sum_c s2[c]*A[c,n] + sum_c shift2[c]*rowsum_weff[c];
            # scale-first so the first matmul issues while the shift path
            # is still on DVE
            t2 = sp.tile([128, 4], F32, name="bn2t")
            mean2, var2, sd2, tn2 = (t2[:, i:i + 1] for i in range(4))
            nc.vector.tensor_scalar(mean2, stg2[:, 0:1], 1.0 / N2, None,
                                    op0=mybir.AluOpType.mult)
            nc.vector.tensor_tensor(var2, mean2, mean2, op=mybir.AluOpType.mult)
            nc.vector.tensor_tensor(var2, stg2[:, 1:2], var2,
                                    op=mybir.AluOpType.subtract)
            nc.scalar.activation(sd2, var2, mybir.ActivationFunctionType.Sqrt,
                                 bias=epst[:, 0:1])
            co2 = sp.tile([128, 3], F32, name="bn2c")
            scale2, shift2, r2 = co2[:, 0:1], co2[:, 1:2], co2[:, 2:3]
            nc.vector.reciprocal(r2, sd2)
            nc.vector.tensor_tensor(scale2, bsb[:, BC_BN2G:BC_BN2G + 1], r2,
                                    op=mybir.AluOpType.mult)
            s2b = sp.tile([128, 1], BF16)
            nc.vector.tensor_copy(s2b[:], scale2)
            zps = cps.tile([1, BL], F32, tag="c1ps")
            nc.tensor.matmul(zps[:], s2b[:], Avb[:], start=True, stop=False)
            nc.vector.tensor_tensor(tn2, mean2, scale2, op=mybir.AluOpType.mult)
            nc.vector.tensor_tensor(shift2, bsb[:, BC_BN2B:BC_BN2B + 1], tn2,
                                    op=mybir.AluOpType.subtract)
            vsh = wp.tile([128, 1], BF16)
            nc.vector.tensor_tensor(vsh[:], shift2, weff[:, 25:26],
                                    op=mybir.AluOpType.mult)
            nc.tensor.matmul(zps[:], vsh[:], ones[:], start=False, stop=True)
            osb = sp.tile([1, BL], F32)
            nc.scalar.activation(osb[:], zps[:],
                                 mybir.ActivationFunctionType.Sigmoid,
                                 bias=bsb[0:1, BC_BEFF:BC_BEFF + 1])
            nc.sync.dma_start(bass.AP(out, 0, [[1, 1], [1, BL]]), osb[:])

    nc.compile()
    return nc


# ----------------------------------------------------------------------------
# host-side input prep
# ----------------------------------------------------------------------------

def _prep_inputs(inputs):
    import ml_dtypes
    f = np.float32
    bf = ml_dtypes.bfloat16
    x = np.asarray(inputs["x"], dtype=f)

    # conv1 patches: [n64, cb4, c128, i14, ki3, j14, kj3]
    xpad = np.zeros((B, 512, 42, 42), dtype=bf)
    xpad[:, :, 1:41, 1:41] = x.astype(bf)
    # -> [k9, cb4, c128, n64, pos196]
    xv = (xpad.reshape(B, 4, 128, 14, 3, 14, 3)
          .transpose(4, 6, 1, 2, 0, 3, 5)        # ki,kj,cb,c,n,i,j
          .reshape(9, 4, 128, B, P1))

    w1 = np.asarray(inputs["conv1_w"], dtype=f)          # [256, 512, 3, 3]
    # [128c, 9k, 4cb, 256m]
    w1p = np.ascontiguousarray(
        w1.reshape(256, 4, 128, 9).transpose(2, 3, 1, 0)).reshape(
            128, 9, 1024).astype(bf)
    w2 = np.asarray(inputs["conv2_w"], dtype=f)          # [128, 256, 3, 3]
    w2p = np.ascontiguousarray(
        w2.reshape(128, 2, 128, 9).transpose(2, 1, 3, 0)).reshape(
            128, 18, 128).astype(bf)
    # boundary-class row sums for the BN1-shift term:
    # class c = a*2+b, a=(i==0) -> ki>=1 only, b=(j==0) -> kj>=1 only
    w2r = w2.reshape(128, 2, 128, 3, 3)                  # c2, cb, c1w, ki, kj
    w2sp = np.zeros((128, 8 * 128), dtype=f)
    for cls in range(4):
        a, b = cls // 2, cls % 2
        kis = slice(1, 3) if a else slice(0, 3)
        kjs = slice(1, 3) if b else slice(0, 3)
        s = w2r[:, :, :, kis, kjs].sum(axis=(3, 4))      # c2, cb, c1w
        for cb in range(2):
            w2sp[:, (cls * 2 + cb) * 128:(cls * 2 + cb + 1) * 128] = s[:, cb].T
    w2sp = w2sp.astype(bf)

    # compose the 12 affine layers (no nonlinearities) into [3200] + scalar
    M = np.asarray(inputs["w14"], dtype=np.float64)      # [1, 2]
    beff = np.asarray(inputs["b14"], dtype=np.float64).copy()  # [1]
    for li in range(13, 2, -1):                          # w13 .. w3
        beff += M @ np.asarray(inputs[f"b{li}"], dtype=np.float64)
        M = M @ np.asarray(inputs[f"w{li}"], dtype=np.float64)
    weff = M.reshape(3200).astype(f)                     # order f = c*25 + ij
    w2d = weff.reshape(128, 25)
    weffp = np.zeros((128, 26), dtype=f)
    weffp[:, 0:25] = w2d
    weffp[:, 25] = w2d.sum(axis=1)
    beff_f = float(beff[0])

    bp = np.zeros((128, 7), dtype=f)
    bp[:, 0:2] = np.asarray(inputs["bn1_g"], dtype=f).reshape(2, 128).T
    bp[:, 2:4] = np.asarray(inputs["bn1_b"], dtype=f).reshape(2, 128).T
    bp[:, 4] = np.asarray(inputs["bn2_g"], dtype=f)
    bp[:, 5] = np.asarray(inputs["bn2_b"], dtype=f)
    bp[0, 6] = beff_f

    in_maps = []
    for r in range(NCORES):
        # [9k, 4cb, 128, 8n, 196] -> [9k, 2half, 128, 4cb, 2ptin, 2n, 196]
        xr = np.ascontiguousarray(
            xv[:, :, :, r * BL:(r + 1) * BL]
            .reshape(9, 4, 128, 2, 2, 2, P1)     # k, cb, c, half, ptin, n2, pos
            .transpose(0, 3, 2, 1, 4, 5, 6)
        ).reshape(9, 2, 128, 4 * 2 * PTW)
        in_maps.append({
            "xprep": xr, "w1p": w1p, "w2p": w2p, "w2sp": w2sp,
            "weffp": weffp, "bprep": bp,
        })
    return in_maps


def kernel(**inputs):
    if "nc" not in _CACHE:
        _CACHE["nc"] = _build()
    nc = _CACHE["nc"]
    in_maps = _prep_inputs(inputs)
    trace = bool(int(os.environ.get("KERNEL_TRACE", "0")))
    if trace:
        try:
            import ntff_shim
            ntff_shim.install()
        except ImportError:
            trace = False
    res = run_bass_kernel_spmd(nc, in_maps, core_ids=list(range(NCORES)),
                               trace=trace)
    _CACHE["last_result"] = res
    return np.concatenate([res.results[r]["out"] for r in range(NCORES)], axis=0)


# revision 41
# speedup vs baseline: 1.1474x; 1.1061x over previous
"""Trainium2 Bass kernel for nn_DomainDiscriminator.

Network: conv(512->256,k3,s3,p1) -> BN -> conv(256->128,k3,s3,p1) -> BN
         -> reshape -> 12-layer MLP (3200->...->1, no nonlinearities) -> sigmoid.
Input x: [64, 512, 40, 40] f32.  Output: [64, 1] f32.

Strategy (8 NeuronCores, pure data parallel, 8 batch per core):
 - stride==kernel==3 convs are non-overlapping patch matmuls. Conv1 patches
   are built host-side (space-to-depth, bf16); conv2 patches are read out of
   SBUF with strided access patterns.
 - conv1 streams x in 18 small (k, half) tiles; the matmul loop is k-major
   within each half (pair of psum tiles per mt) so each weight tile is loaded
   once per 2 matmuls and the first matmul starts as soon as the first 392KB
   tile lands.
 - Training-mode BN: conv biases are absorbed exactly by BN; per-channel batch
   stats are computed per-psum-tile during conv1 (on DVE/ACT, which idle),
   exchanged with tiny 2KB AllGathers, tree-summed on chip.
 - ACT tables (Square/Sqrt/Sigmoid) are preloaded off the critical path;
   the BN2 AllGather is triggered before the weff partial matvec.
 - The 12 linear layers compose on the host (fp64) into a single [3200]
   vector + scalar bias; the device finishes with two tiny matmuls + sigmoid.
"""

import os
import sys

sys.path.insert(0, "/opt/trn_rl_repo")

import numpy as np

import concourse.bass as bass
import concourse.mybir as mybir
import concourse.tile as tile
from concourse import bacc
from concourse.bass_utils import run_bass_kernel_spmd

F32 = mybir.dt.float32
BF16 = mybir.dt.bfloat16

NCORES = 8
BL = 8              # batch per core
B = 64              # full batch
EPS = 1e-5

P1 = 196            # 14*14 conv1 positions
P2 = 25             # 5*5 conv2 positions
PTW = 2 * P1        # 392 cols per conv1 psum tile (2 batch)
N1 = float(B * P1)  # BN1 stat count
N2 = float(B * P2)  # BN2 stat count

_CACHE = {}

KIJ9 = [(ki, kj) for ki in range(3) for kj in range(3)]
# conv2 im2col: per (ki,kj) a full zero-padded 5*5*8 block in h1sb
BLK = 200
H1W = 9 * BLK       # 1800 cols per h1 tile


# ----------------------------------------------------------------------------
# device program
# ----------------------------------------------------------------------------

def _emit_casts(nc, work, h1sb):
    """psum -> h1sb bf16 im2col-block copies for one conv1 half."""
    half, ps = work
    for ptin in range(2):
        pt = half * 2 + ptin
        for mt in range(2):
            pr = ps[(ptin, mt)][:].rearrange("p (n i j) -> p n i j",
                                             n=2, i=14, j=14)
            for kidx, (ki, kj) in enumerate(KIJ9):
                ilo, icnt = (1, 4) if ki == 0 else (0, 5)
                jlo, jcnt = (1, 4) if kj == 0 else (0, 5)
                srcv = pr[:, :, 3 * ilo + ki - 1:14:3,
                          3 * jlo + kj - 1:14:3].transpose([0, 2, 3, 1])
                off = kidx * BLK + (ilo * 5 + jlo) * 8
                dstv = bass.AP(
                    h1sb[mt].tensor,
                    h1sb[mt].offset + off + 2 * pt,
                    [list(h1sb[mt].ap[0]), [40, icnt], [8, jcnt], [1, 2]])
                nc.vector.tensor_copy(dstv, srcv)


def _build():
    nc = bacc.Bacc("TRN2", target_bir_lowering=False, debug=False,
                   enable_asserts=True, num_devices=NCORES)

    # xprep: [9 kij, 2 half, 128, 4cb * 2ptin * 392]  (cb-major, ptin, cols)
    xprep = nc.dram_tensor("xprep", [9, 2, 128, 4 * 2 * PTW], BF16,
                           kind="ExternalInput")
    # w1p: [128, 9 kij, 4 cb, 256 m]
    w1p = nc.dram_tensor("w1p", [128, 9, 4 * 256], BF16, kind="ExternalInput")
    w2p = nc.dram_tensor("w2p", [128, 18, 128], BF16, kind="ExternalInput")
    # conv2 boundary-class row sums: [(class4, cb2) blocks][c1w, c2]
    w2sp = nc.dram_tensor("w2sp", [128, 8 * 128], BF16, kind="ExternalInput")
    weffp = nc.dram_tensor("weffp", [128, 26], F32, kind="ExternalInput")
    bprep = nc.dram_tensor("bprep", [128, 7], F32, kind="ExternalInput")
    out = nc.dram_tensor("out", [BL, 1], F32, kind="ExternalOutput")

    # bprep columns: bn1_g (2), bn1_b (2), bn2_g, bn2_b, beff(row 0)
    BC_BN1G, BC_BN1B, BC_BN2G, BC_BN2B, BC_BEFF = 0, 2, 4, 5, 6

    ISQ1 = 1.0 / np.sqrt(N1)   # Square-accum scale so accum = sum(h^2)/N1
    ISQ2 = 1.0 / np.sqrt(N2)

    with tile.TileContext(nc) as tc:
        with tc.tile_pool(name="wp", bufs=1) as wp, \
             tc.tile_pool(name="xp", bufs=7) as xp, \
             tc.tile_pool(name="hp", bufs=1) as hp, \
             tc.tile_pool(name="sp", bufs=1) as sp, \
             tc.tile_pool(name="cps", bufs=8, space="PSUM") as cps, \
             tc.tile_pool(name="dram", bufs=1, space="DRAM") as dram:

            # ---------------- first loads ------------------------------
            # x tiles stream on the Sync HWDGE queue; all weights ride the
            # Scalar queue so the x stream never stalls behind them.
            w1sb = wp.tile([128, 9 * 1024], BF16)
            xt = {}
            for half in range(2):
                for k in range(9):
                    xt[(k, half)] = xp.tile([128, 8 * PTW], BF16,
                                            name=f"xt{k}_{half}", tag="xt")
            # the very first matmul needs only cb0's x slice and k0/cb0's
            # weights — gate it on ~260KB of cold DMA, stream the rest behind
            w1r = w1p.ap().rearrange("p a b -> p (a b)")
            nc.sync.dma_start(xt[(0, 0)][:, 0:2 * PTW],
                              xprep.ap()[0, 0][:, 0:2 * PTW])
            nc.sync.dma_start(w1sb[:, 0:256], w1r[:, 0:256])
            nc.sync.dma_start(xt[(0, 0)][:, 2 * PTW:4 * PTW],
                              xprep.ap()[0, 0][:, 2 * PTW:4 * PTW])
            nc.sync.dma_start(w1sb[:, 256:1024], w1r[:, 256:1024])
            nc.sync.dma_start(xt[(0, 0)][:, 4 * PTW:],
                              xprep.ap()[0, 0][:, 4 * PTW:])

            # ncfw warm-up: a tiny AllGather nobody consumes; hides the
            # TOPSP cold-start under conv1
            warm_in = dram.tile([1, 4], F32)
            warm_out = dram.tile([NCORES, 1, 4], F32, addr_space="Shared")
            dummy = sp.tile([1, 8], F32)
            nc.gpsimd.memset(dummy[:], 0.0)
            epst = sp.tile([128, 1], F32)
            nc.gpsimd.memset(epst[:], EPS)
            nc.scalar.dma_start(warm_in[:], dummy[:, 0:4])
            nc.gpsimd.collective_compute(
                "AllGather", mybir.AluOpType.bypass,
                replica_groups=[list(range(NCORES))],
                ins=[warm_in.opt()], outs=[warm_out.opt()])
            # ACT Square table preload while ACT is idle
            nc.scalar.activation(dummy[:, 4:5], dummy[:, 5:6],
                                 mybir.ActivationFunctionType.Square)

            # remaining streamed loads, interleaved with compute demand order
            for k in range(1, 9):
                nc.sync.dma_start(xt[(k, 0)][:], xprep.ap()[k, 0])
                nc.sync.dma_start(w1sb[:, k * 1024:(k + 1) * 1024],
                                  w1r[:, k * 1024:(k + 1) * 1024])
            for k in range(9):
                nc.sync.dma_start(xt[(k, 1)][:], xprep.ap()[k, 1])
            w2sb = wp.tile([128, 18 * 128], BF16)
            nc.sync.dma_start(w2sb[:], w2p.ap().rearrange("p a b -> p (a b)"))
            w2s_sb = wp.tile([128, 8 * 128], BF16)
            nc.sync.dma_start(w2s_sb[:], w2sp.ap())
            weff = wp.tile([128, 26], F32)
            nc.sync.dma_start(weff[:], weffp.ap())
            bsb = wp.tile([128, 7], F32)
            nc.sync.dma_start(bsb[:], bprep.ap())

            # ---------------- conv1 (k-major per half) ------------------
            scratch = sp.tile([128, PTW], F32)
            # h1 tiles hold RAW conv1 output in zero-padded 5*5*8 blocks
            # per (ki,kj); border slots stay zero (memset once)
            h1sb = [hp.tile([128, H1W], BF16, name=f"h1_{mt}")
                    for mt in range(2)]
            for mt in range(2):
                nc.gpsimd.memset(h1sb[mt][:], 0.0)
            # per-psum-tile stats: [128, 4pt * (2kind*2mt)]; kind0=sum, 1=sumsq
            stt = sp.tile([128, 16], F32)

            cast_work = []
            for half in range(2):
                ps = {}
                for ptin in range(2):
                    for mt in range(2):
                        ps[(ptin, mt)] = cps.tile([128, PTW], F32,
                                                  name=f"c1ps{half}{ptin}{mt}",
                                                  tag="c1ps")
                for k in range(9):
                    xk = xt[(k, half)][:].rearrange(
                        "p (c t w) -> p c t w", c=4, t=2)
                    for cb in range(4):
                        for mt in range(2):
                            lhsT = w1sb[:, (k * 4 + cb) * 256 + mt * 128:
                                        (k * 4 + cb) * 256 + (mt + 1) * 128]
                            for ptin in range(2):
                                nc.tensor.matmul(
                                    ps[(ptin, mt)][:], lhsT, xk[:, cb, ptin],
                                    start=(k == 0 and cb == 0),
                                    stop=(k == 8 and cb == 3))
                # stats first — they gate the AllGather trigger; the casts
                # drain later (half0's under half1's matmuls, half1's under
                # the collective). Sums on DVE, sum(h^2)/N1 on ACT.
                for ptin in range(2):
                    pt = half * 2 + ptin
                    for mt in range(2):
                        p = ps[(ptin, mt)]
                        nc.vector.reduce_sum(stt[:, pt * 4 + mt:pt * 4 + mt + 1],
                                             p[:], axis=mybir.AxisListType.X)
                        nc.scalar.activation(scratch[:], p[:],
                                             mybir.ActivationFunctionType.Square,
                                             scale=ISQ1,
                                             accum_out=stt[:, pt * 4 + 2 + mt:
                                                           pt * 4 + 3 + mt])
                cast_work.append((half, ps))
                if half == 0:
                    _emit_casts(nc, cast_work.pop(), h1sb)

            # ---------------- BN1 stats combine + AllGather -------------
            # combine 4 pt blocks: [128, 4pt, 4] -> [128, 4]
            nc.vector.tensor_tensor(stt[:, 0:8], stt[:, 0:8], stt[:, 8:16],
                                    op=mybir.AluOpType.add)
            nc.vector.tensor_tensor(stt[:, 0:4], stt[:, 0:4], stt[:, 4:8],
                                    op=mybir.AluOpType.add)
            st1 = stt[:, 0:4]    # [S_mt0, S_mt1, Q_mt0, Q_mt1]; Q pre-div N1
            bn1_in = dram.tile([128, 4], F32)
            bn1_out = dram.tile([NCORES, 128, 4], F32, addr_space="Shared")
            nc.scalar.dma_start(bn1_in[:], st1)
            nc.gpsimd.collective_compute(
                "AllGather", mybir.AluOpType.bypass,
                replica_groups=[list(range(NCORES))],
                ins=[bn1_in.opt()], outs=[bn1_out.opt()])
            # Sqrt table preload during the AllGather wait; the stt read
            # keeps it ordered after the stats Squares, scale=0 + eps bias
            # keeps the argument in sqrt's valid range
            nc.scalar.activation(dummy[:, 4:5], stt[0:1, 0:1],
                                 mybir.ActivationFunctionType.Sqrt,
                                 scale=0.0, bias=epst[0:1, 0:1])
            # half1's im2col casts drain during the AllGather
            _emit_casts(nc, cast_work.pop(), h1sb)
            stg = sp.tile([128, NCORES * 4], F32)
            nc.scalar.dma_start(
                stg[:].rearrange("p (r t) -> p r t", r=NCORES),
                bass.AP(bn1_out.tensor, 0, [[4, 128], [128 * 4, NCORES], [1, 4]]))
            stgr = stg[:].rearrange("p (r t) -> p r t", r=NCORES)
            for halfn in (4, 2, 1):
                nc.vector.tensor_tensor(
                    stgr[:, 0:halfn], stgr[:, 0:halfn],
                    stgr[:, halfn:2 * halfn], op=mybir.AluOpType.add)

            # BN1 coeffs, scale-first so conv2's weight scaling can start
            # before the shift path finishes. BN1 is folded into conv2:
            # the per-input-channel scale goes into w2 (h1sb's zero padding
            # must stay zero), the shift's contribution is a per-boundary-
            # class constant from 8 tiny matmuls against host-precomputed
            # class row sums.
            t1 = sp.tile([128, 8], F32, name="bn1t")
            mean1, var1, sd1, tn1 = (t1[:, i * 2:(i + 1) * 2] for i in range(4))
            nc.vector.tensor_scalar(mean1, stg[:, 0:2], 1.0 / N1, None,
                                    op0=mybir.AluOpType.mult)
            nc.vector.tensor_tensor(var1, mean1, mean1, op=mybir.AluOpType.mult)
            nc.vector.tensor_tensor(var1, stg[:, 2:4], var1,
                                    op=mybir.AluOpType.subtract)
            nc.scalar.activation(sd1, var1, mybir.ActivationFunctionType.Sqrt,
                                 bias=epst[:, 0:1])
            co1 = sp.tile([128, 6], F32, name="bn1c")
            scale1, shift1, r1 = co1[:, 0:2], co1[:, 2:4], co1[:, 4:6]
            nc.vector.reciprocal(r1, sd1)
            nc.vector.tensor_tensor(scale1, bsb[:, BC_BN1G:BC_BN1G + 2], r1,
                                    op=mybir.AluOpType.mult)
            for cb2 in range(2):
                nc.vector.tensor_scalar(
                    w2sb[:, cb2 * 1152:(cb2 + 1) * 1152],
                    w2sb[:, cb2 * 1152:(cb2 + 1) * 1152],
                    scale1[:, cb2:cb2 + 1], None, op0=mybir.AluOpType.mult)
            nc.vector.tensor_tensor(tn1, mean1, scale1, op=mybir.AluOpType.mult)
            nc.vector.tensor_tensor(shift1, bsb[:, BC_BN1B:BC_BN1B + 2], tn1,
                                    op=mybir.AluOpType.subtract)
            shift1b = sp.tile([128, 2], BF16)
            nc.vector.tensor_copy(shift1b[:], shift1)
            Tm = cps.tile([128, 4], F32, tag="c1ps")
            for cls in range(4):
                for cb2 in range(2):
                    nc.tensor.matmul(Tm[:, cls:cls + 1],
                                     w2s_sb[:, (cls * 2 + cb2) * 128:
                                            (cls * 2 + cb2 + 1) * 128],
                                     shift1b[:, cb2:cb2 + 1],
                                     start=(cb2 == 0), stop=(cb2 == 1),
                                     skip_group_check=True)
            TmS = sp.tile([128, 4], F32)
            nc.vector.tensor_copy(TmS[:], Tm[:])
            # broadcast the class constants into a full [128,200] map while
            # conv2 runs, so the post-conv2 merge is a single add
            Tmap = sp.tile([128, BL * P2], F32)
            Tv = Tmap[:].rearrange("p (i j n) -> p i j n", i=5, j=5, n=BL)
            for cls, sl in ((3, (slice(0, 1), slice(0, 1))),
                            (2, (slice(0, 1), slice(1, 5))),
                            (1, (slice(1, 5), slice(0, 1))),
                            (0, (slice(1, 5), slice(1, 5)))):
                ni = sl[0].stop - sl[0].start
                nj = (sl[1].stop - sl[1].start) * BL
                src = TmS[:, cls:cls + 1, None].to_broadcast([128, ni, nj])
                nc.vector.tensor_copy(
                    Tv[:, sl[0], sl[1]].rearrange("p a b c -> p a (b c)"), src)

            # ---------------- conv2 (one 18-matmul chain) ---------------
            c2p = cps.tile([128, P2 * BL], F32, name="c2p", tag="c1ps")
            for idx in range(18):
                cb2, kidx = idx // 9, idx % 9
                lhsT = w2sb[:, idx * 128:(idx + 1) * 128]
                nc.tensor.matmul(c2p[:], lhsT,
                                 h1sb[cb2][:, kidx * BLK:(kidx + 1) * BLK],
                                 start=(idx == 0), stop=(idx == 17))
            # psum -> sbuf bf16, adding the BN1-shift map in one pass
            c2sb = sp.tile([128, BL * P2], BF16)
            nc.vector.tensor_tensor(c2sb[:], Tmap[:], c2p[:],
                                    op=mybir.AluOpType.add)

            # ---------------- BN2 stats + AllGather (before matvec) -----
            st2l = sp.tile([128, 2], F32)
            nc.vector.reduce_sum(st2l[:, 0:1], c2sb[:], axis=mybir.AxisListType.X)
            sc2 = sp.tile([128, BL * P2], F32)
            nc.scalar.activation(sc2[:], c2sb[:],
                                 mybir.ActivationFunctionType.Square,
                                 scale=ISQ2, accum_out=st2l[:, 1:2])
            bn2_in = dram.tile([128, 2], F32)
            bn2_out = dram.tile([NCORES, 128, 2], F32, addr_space="Shared")
            nc.scalar.dma_start(bn2_in[:], st2l[:])
            nc.gpsimd.collective_compute(
                "AllGather", mybir.AluOpType.bypass,
                replica_groups=[list(range(NCORES))],
                ins=[bn2_in.opt()], outs=[bn2_out.opt()])

            # during the AllGather: weff partial matvec + Sigmoid table +
            # the pieces of the finish that don't need stats
            mvt = sp.tile([128, P2 * BL], F32)
            wb = weff[:, 0:25, None].to_broadcast([128, 25, BL])
            nc.vector.tensor_tensor(
                mvt[:].rearrange("p (i n) -> p i n", i=P2),
                c2sb[:].rearrange("p (i n) -> p i n", i=P2), wb,
                op=mybir.AluOpType.mult)
            Av = sp.tile([128, BL], F32)
            nc.vector.reduce_sum(Av[:], mvt[:].rearrange("p (i n) -> p n i", i=P2),
                                 axis=mybir.AxisListType.X)
            Avb = sp.tile([128, BL], BF16)
            nc.vector.tensor_copy(Avb[:], Av[:])
            nc.scalar.activation(dummy[:, 4:5], st2l[0:1, 0:1],
                                 mybir.ActivationFunctionType.Sigmoid)
            ones = wp.tile([128, BL], BF16)
            nc.gpsimd.memset(ones[:], 1.0)

            stg2 = sp.tile([128, NCORES * 2], F32)
            nc.scalar.dma_start(
                stg2[:].rearrange("p (r t) -> p r t", r=NCORES),
                bass.AP(bn2_out.tensor, 0, [[2, 128], [128 * 2, NCORES], [1, 2]]))
            stg2r = stg2[:].rearrange("p (r t) -> p r t", r=NCORES)
            for halfn in (4, 2, 1):
                nc.vector.tensor_tensor(
                    stg2r[:, 0:halfn], stg2r[:, 0:halfn],
                    stg2r[:, halfn:2 * halfn], op=mybir.AluOpType.add)

            # ---------------- BN2 coeffs + collapsed MLP finish ---------
            # z[n] = sum_c s2[c]*A[c,n] + sum_c shift2[c]*rowsum_weff[c];
            # scale-first so the first matmul issues while the shift path
            # is still on DVE
            t2 = sp.tile([128, 4], F32, name="bn2t")
            mean2, var2, sd2, tn2 = (t2[:, i:i + 1] for i in range(4))
            nc.vector.tensor_scalar(mean2, stg2[:, 0:1], 1.0 / N2, None,
                                    op0=mybir.AluOpType.mult)
            nc.vector.tensor_tensor(var2, mean2, mean2, op=mybir.AluOpType.mult)
            nc.vector.tensor_tensor(var2, stg2[:, 1:2], var2,
                                    op=mybir.AluOpType.subtract)
            nc.scalar.activation(sd2, var2, mybir.ActivationFunctionType.Sqrt,
                                 bias=epst[:, 0:1])
            co2 = sp.tile([128, 3], F32, name="bn2c")
            scale2, shift2, r2 = co2[:, 0:1], co2[:, 1:2], co2[:, 2:3]
            nc.vector.reciprocal(r2, sd2)
            nc.vector.tensor_tensor(scale2, bsb[:, BC_BN2G:BC_BN2G + 1], r2,
                                    op=mybir.AluOpType.mult)
            s2b = sp.tile([128, 1], BF16)
            nc.vector.tensor_copy(s2b[:], scale2)
            zps = cps.tile([1, BL], F32, tag="c1ps")
            nc.tensor.matmul(zps[:], s2b[:], Avb[:], start=True, stop=False)
            nc.vector.tensor_tensor(tn2, mean2, scale2, op=mybir.AluOpType.mult)
            nc.vector.tensor_tensor(shift2, bsb[:, BC_BN2B:BC_BN2B + 1], tn2,
                                    op=mybir.AluOpType.subtract)
            vsh = wp.tile([128, 1], BF16)
            nc.vector.tensor_tensor(vsh[:], shift2, weff[:, 25:26],
                                    op=mybir.AluOpType.mult)
            nc.tensor.matmul(zps[:], vsh[:], ones[:], start=False, stop=True)
            osb = sp.tile([1, BL], F32)
            nc.scalar.activation(osb[:], zps[:],
                                 mybir.ActivationFunctionType.Sigmoid,
                                 bias=bsb[0:1, BC_BEFF:BC_BEFF + 1])
            nc.sync.dma_start(bass.AP(out, 0, [[1, 1], [1, BL]]), osb[:])

    nc.compile()
    return nc


# ----------------------------------------------------------------------------
# host-side input prep
# ----------------------------------------------------------------------------

def _prep_inputs(inputs):
    import ml_dtypes
    f = np.float32
    bf = ml_dtypes.bfloat16
    x = np.asarray(inputs["x"], dtype=f)

    # conv1 patches: [n64, cb4, c128, i14, ki3, j14, kj3]
    xpad = np.zeros((B, 512, 42, 42), dtype=bf)
    xpad[:, :, 1:41, 1:41] = x.astype(bf)
    # -> [k9, cb4, c128, n64, pos196]
    xv = (xpad.reshape(B, 4, 128, 14, 3, 14, 3)
          .transpose(4, 6, 1, 2, 0, 3, 5)        # ki,kj,cb,c,n,i,j
          .reshape(9, 4, 128, B, P1))

    w1 = np.asarray(inputs["conv1_w"], dtype=f)          # [256, 512, 3, 3]
    # [128c, 9k, 4cb, 256m]
    w1p = np.ascontiguousarray(
        w1.reshape(256, 4, 128, 9).transpose(2, 3, 1, 0)).reshape(
            128, 9, 1024).astype(bf)
    w2 = np.asarray(inputs["conv2_w"], dtype=f)          # [128, 256, 3, 3]
    w2p = np.ascontiguousarray(
        w2.reshape(128, 2, 128, 9).transpose(2, 1, 3, 0)).reshape(
            128, 18, 128).astype(bf)
    # boundary-class row sums for the BN1-shift term:
    # class c = a*2+b, a=(i==0) -> ki>=1 only, b=(j==0) -> kj>=1 only
    w2r = w2.reshape(128, 2, 128, 3, 3)                  # c2, cb, c1w, ki, kj
    w2sp = np.zeros((128, 8 * 128), dtype=f)
    for cls in range(4):
        a, b = cls // 2, cls % 2
        kis = slice(1, 3) if a else slice(0, 3)
        kjs = slice(1, 3) if b else slice(0, 3)
        s = w2r[:, :, :, kis, kjs].sum(axis=(3, 4))      # c2, cb, c1w
        for cb in range(2):
            w2sp[:, (cls * 2 + cb) * 128:(cls * 2 + cb + 1) * 128] = s[:, cb].T
    w2sp = w2sp.astype(bf)

    # compose the 12 affine layers (no nonlinearities) into [3200] + scalar
    M = np.asarray(inputs["w14"], dtype=np.float64)      # [1, 2]
    beff = np.asarray(inputs["b14"], dtype=np.float64).copy()  # [1]
    for li in range(13, 2, -1):                          # w13 .. w3
        beff += M @ np.asarray(inputs[f"b{li}"], dtype=np.float64)
        M = M @ np.asarray(inputs[f"w{li}"], dtype=np.float64)
    weff = M.reshape(3200).astype(f)                     # order f = c*25 + ij
    w2d = weff.reshape(128, 25)
    weffp = np.zeros((128, 26), dtype=f)
    weffp[:, 0:25] = w2d
    weffp[:, 25] = w2d.sum(axis=1)
    beff_f = float(beff[0])

    bp = np.zeros((128, 7), dtype=f)
    bp[:, 0:2] = np.asarray(inputs["bn1_g"], dtype=f).reshape(2, 128).T
    bp[:, 2:4] = np.asarray(inputs["bn1_b"], dtype=f).reshape(2, 128).T
    bp[:, 4] = np.asarray(inputs["bn2_g"], dtype=f)
    bp[:, 5] = np.asarray(inputs["bn2_b"], dtype=f)
    bp[0, 6] = beff_f

    in_maps = []
    for r in range(NCORES):
        # [9k, 4cb, 128, 8n, 196] -> [9k, 2half, 128, 4cb, 2ptin, 2n, 196]
        xr = np.ascontiguousarray(
            xv[:, :, :, r * BL:(r + 1) * BL]
            .reshape(9, 4, 128, 2, 2, 2, P1)     # k, cb, c, half, ptin, n2, pos
            .transpose(0, 3, 2, 1, 4, 5, 6)
        ).reshape(9, 2, 128, 4 * 2 * PTW)
        in_maps.append({
            "xprep": xr, "w1p": w1p, "w2p": w2p, "w2sp": w2sp,
            "weffp": weffp, "bprep": bp,
        })
    return in_maps


def kernel(**inputs):
    if "nc" not in _CACHE:
        _CACHE["nc"] = _build()
    nc = _CACHE["nc"]
    in_maps = _prep_inputs(inputs)
    trace = bool(int(os.environ.get("KERNEL_TRACE", "0")))
    if trace:
        try:
            import ntff_shim
            ntff_shim.install()
        except ImportError:
            trace = False
    res = run_bass_kernel_spmd(nc, in_maps, core_ids=list(range(NCORES)),
                               trace=trace)
    _CACHE["last_result"] = res
    return np.concatenate([res.results[r]["out"] for r in range(NCORES)], axis=0)


# revision 43
# speedup vs baseline: 1.1599x; 1.0109x over previous
"""Trainium2 Bass kernel for nn_DomainDiscriminator.

Network: conv(512->256,k3,s3,p1) -> BN -> conv(256->128,k3,s3,p1) -> BN
         -> reshape -> 12-layer MLP (3200->...->1, no nonlinearities) -> sigmoid.
Input x: [64, 512, 40, 40] f32.  Output: [64, 1] f32.

Strategy (8 NeuronCores, pure data parallel, 8 batch per core):
 - stride==kernel==3 convs are non-overlapping patch matmuls. Conv1 patches
   are built host-side (space-to-depth, bf16); conv2 patches are read out of
   SBUF with strided access patterns.
 - conv1 streams x in 18 small (k, half) tiles; the matmul loop is k-major
   within each half (pair of psum tiles per mt) so each weight tile is loaded
   once per 2 matmuls and the first matmul starts as soon as the first 392KB
   tile lands.
 - Training-mode BN: conv biases are absorbed exactly by BN; per-channel batch
   stats are computed per-psum-tile during conv1 (on DVE/ACT, which idle),
   exchanged with tiny 2KB AllGathers, tree-summed on chip.
 - ACT tables (Square/Sqrt/Sigmoid) are preloaded off the critical path;
   the BN2 AllGather is triggered before the weff partial matvec.
 - The 12 linear layers compose on the host (fp64) into a single [3200]
   vector + scalar bias; the device finishes with two tiny matmuls + sigmoid.
"""

import os
import sys

sys.path.insert(0, "/opt/trn_rl_repo")

import numpy as np

import concourse.bass as bass
import concourse.mybir as mybir
import concourse.tile as tile
from concourse import bacc
from concourse.bass_utils import run_bass_kernel_spmd

F32 = mybir.dt.float32
BF16 = mybir.dt.bfloat16

NCORES = 8
BL = 8              # batch per core
B = 64              # full batch
EPS = 1e-5

P1 = 196            # 14*14 conv1 positions
P2 = 25             # 5*5 conv2 positions
PTW = 2 * P1        # 392 cols per conv1 psum tile (2 batch)
N1 = float(B * P1)  # BN1 stat count
N2 = float(B * P2)  # BN2 stat count

_CACHE = {}

KIJ9 = [(ki, kj) for ki in range(3) for kj in range(3)]
# conv2 im2col: per (ki,kj) a full zero-padded 5*5*8 block in h1sb
BLK = 200
H1W = 9 * BLK       # 1800 cols per h1 tile


# ----------------------------------------------------------------------------
# device program
# ----------------------------------------------------------------------------

def _emit_casts(nc, work, h1sb):
    """psum -> h1sb bf16 im2col-block copies for one conv1 half."""
    half, ps = work
    for ptin in range(2):
        pt = half * 2 + ptin
        for mt in range(2):
            pr = ps[(ptin, mt)][:].rearrange("p (n i j) -> p n i j",
                                             n=2, i=14, j=14)
            for kidx, (ki, kj) in enumerate(KIJ9):
                ilo, icnt = (1, 4) if ki == 0 else (0, 5)
                jlo, jcnt = (1, 4) if kj == 0 else (0, 5)
                srcv = pr[:, :, 3 * ilo + ki - 1:14:3,
                          3 * jlo + kj - 1:14:3].transpose([0, 2, 3, 1])
                off = kidx * BLK + (ilo * 5 + jlo) * 8
                dstv = bass.AP(
                    h1sb[mt].tensor,
                    h1sb[mt].offset + off + 2 * pt,
                    [list(h1sb[mt].ap[0]), [40, icnt], [8, jcnt], [1, 2]])
                nc.vector.tensor_copy(dstv, srcv)


def _build():
    nc = bacc.Bacc("TRN2", target_bir_lowering=False, debug=False,
                   enable_asserts=True, num_devices=NCORES)

    # xprep: [9 kij, 2 half, 128, 4cb * 2ptin * 392]  (cb-major, ptin, cols)
    xprep = nc.dram_tensor("xprep", [9, 2, 128, 4 * 2 * PTW], BF16,
                           kind="ExternalInput")
    # w1p: [128, 9 kij, 4 cb, 256 m]
    w1p = nc.dram_tensor("w1p", [128, 9, 4 * 256], BF16, kind="ExternalInput")
    w2p = nc.dram_tensor("w2p", [128, 18, 128], BF16, kind="ExternalInput")
    # conv2 boundary-class row sums: [(class4, cb2) blocks][c1w, c2]
    w2sp = nc.dram_tensor("w2sp", [128, 8 * 128], BF16, kind="ExternalInput")
    weffp = nc.dram_tensor("weffp", [128, 26], F32, kind="ExternalInput")
    bprep = nc.dram_tensor("bprep", [128, 7], F32, kind="ExternalInput")
    out = nc.dram_tensor("out", [BL, 1], F32, kind="ExternalOutput")

    # bprep columns: bn1_g (2), bn1_b (2), bn2_g, bn2_b, beff(row 0)
    BC_BN1G, BC_BN1B, BC_BN2G, BC_BN2B, BC_BEFF = 0, 2, 4, 5, 6

    ISQ1 = 1.0 / np.sqrt(N1)   # Square-accum scale so accum = sum(h^2)/N1
    ISQ2 = 1.0 / np.sqrt(N2)

    with tile.TileContext(nc) as tc:
        with tc.tile_pool(name="wp", bufs=1) as wp, \
             tc.tile_pool(name="xp", bufs=9) as xp, \
             tc.tile_pool(name="hp", bufs=1) as hp, \
             tc.tile_pool(name="sp", bufs=1) as sp, \
             tc.tile_pool(name="cps", bufs=8, space="PSUM") as cps, \
             tc.tile_pool(name="dram", bufs=1, space="DRAM") as dram:

            # ---------------- first loads ------------------------------
            # x tiles stream on the Sync HWDGE queue; all weights ride the
            # Scalar queue so the x stream never stalls behind them.
            w1sb = wp.tile([128, 9 * 1024], BF16)
            xt = {}
            for half in range(2):
                for k in range(9):
                    xt[(k, half)] = xp.tile([128, 8 * PTW], BF16,
                                            name=f"xt{k}_{half}", tag="xt")
            # the very first matmul needs only cb0/ptin0's x slice and
            # k0/cb0's weights — gate it on ~160KB of cold DMA, stream the
            # rest behind in demand order
            w1r = w1p.ap().rearrange("p a b -> p (a b)")
            nc.sync.dma_start(xt[(0, 0)][:, 0:PTW],
                              xprep.ap()[0, 0][:, 0:PTW])
            nc.sync.dma_start(w1sb[:, 0:256], w1r[:, 0:256])
            nc.sync.dma_start(xt[(0, 0)][:, PTW:2 * PTW],
                              xprep.ap()[0, 0][:, PTW:2 * PTW])
            nc.sync.dma_start(xt[(0, 0)][:, 2 * PTW:4 * PTW],
                              xprep.ap()[0, 0][:, 2 * PTW:4 * PTW])
            nc.sync.dma_start(w1sb[:, 256:1024], w1r[:, 256:1024])
            nc.sync.dma_start(xt[(0, 0)][:, 4 * PTW:],
                              xprep.ap()[0, 0][:, 4 * PTW:])

            # ncfw warm-up: a tiny AllGather nobody consumes; hides the
            # TOPSP cold-start under conv1
            warm_in = dram.tile([1, 4], F32)
            warm_out = dram.tile([NCORES, 1, 4], F32, addr_space="Shared")
            dummy = sp.tile([1, 8], F32)
            nc.gpsimd.memset(dummy[:], 0.0)
            epst = sp.tile([128, 1], F32)
            nc.gpsimd.memset(epst[:], EPS)
            nc.scalar.dma_start(warm_in[:], dummy[:, 0:4])
            nc.gpsimd.collective_compute(
                "AllGather", mybir.AluOpType.bypass,
                replica_groups=[list(range(NCORES))],
                ins=[warm_in.opt()], outs=[warm_out.opt()])
            # ACT Square table preload while ACT is idle
            nc.scalar.activation(dummy[:, 4:5], dummy[:, 5:6],
                                 mybir.ActivationFunctionType.Square)

            # remaining streamed loads, interleaved with compute demand order
            for k in range(1, 9):
                nc.sync.dma_start(xt[(k, 0)][:], xprep.ap()[k, 0])
                nc.sync.dma_start(w1sb[:, k * 1024:(k + 1) * 1024],
                                  w1r[:, k * 1024:(k + 1) * 1024])
            for k in range(9):
                nc.sync.dma_start(xt[(k, 1)][:], xprep.ap()[k, 1])
            w2sb = wp.tile([128, 18 * 128], BF16)
            nc.sync.dma_start(w2sb[:], w2p.ap().rearrange("p a b -> p (a b)"))
            w2s_sb = wp.tile([128, 8 * 128], BF16)
            nc.sync.dma_start(w2s_sb[:], w2sp.ap())
            weff = wp.tile([128, 26], F32)
            nc.sync.dma_start(weff[:], weffp.ap())
            bsb = wp.tile([128, 7], F32)
            nc.sync.dma_start(bsb[:], bprep.ap())

            # ---------------- conv1 (k-major per half) ------------------
            scratch = sp.tile([128, PTW], F32)
            # h1 tiles hold RAW conv1 output in zero-padded 5*5*8 blocks
            # per (ki,kj); border slots stay zero (memset once)
            h1sb = [hp.tile([128, H1W], BF16, name=f"h1_{mt}")
                    for mt in range(2)]
            for mt in range(2):
                nc.gpsimd.memset(h1sb[mt][:], 0.0)
            # per-psum-tile stats: [128, 4pt * (2kind*2mt)]; kind0=sum, 1=sumsq
            stt = sp.tile([128, 16], F32)

            cast_work = []
            for half in range(2):
                ps = {}
                for ptin in range(2):
                    for mt in range(2):
                        ps[(ptin, mt)] = cps.tile([128, PTW], F32,
                                                  name=f"c1ps{half}{ptin}{mt}",
                                                  tag="c1ps")
                for k in range(9):
                    xk = xt[(k, half)][:].rearrange(
                        "p (c t w) -> p c t w", c=4, t=2)
                    for cb in range(4):
                        for mt in range(2):
                            lhsT = w1sb[:, (k * 4 + cb) * 256 + mt * 128:
                                        (k * 4 + cb) * 256 + (mt + 1) * 128]
                            for ptin in range(2):
                                nc.tensor.matmul(
                                    ps[(ptin, mt)][:], lhsT, xk[:, cb, ptin],
                                    start=(k == 0 and cb == 0),
                                    stop=(k == 8 and cb == 3))
                # stats first — they gate the AllGather trigger; the casts
                # drain later (half0's under half1's matmuls, half1's under
                # the collective). Sums on DVE, sum(h^2)/N1 on ACT.
                for ptin in range(2):
                    pt = half * 2 + ptin
                    for mt in range(2):
                        p = ps[(ptin, mt)]
                        nc.vector.reduce_sum(stt[:, pt * 4 + mt:pt * 4 + mt + 1],
                                             p[:], axis=mybir.AxisListType.X)
                        nc.scalar.activation(scratch[:], p[:],
                                             mybir.ActivationFunctionType.Square,
                                             scale=ISQ1,
                                             accum_out=stt[:, pt * 4 + 2 + mt:
                                                           pt * 4 + 3 + mt])
                cast_work.append((half, ps))
                if half == 0:
                    _emit_casts(nc, cast_work.pop(), h1sb)

            # ---------------- BN1 stats combine + AllGather -------------
            # combine 4 pt blocks: [128, 4pt, 4] -> [128, 4]
            nc.vector.tensor_tensor(stt[:, 0:8], stt[:, 0:8], stt[:, 8:16],
                                    op=mybir.AluOpType.add)
            nc.vector.tensor_tensor(stt[:, 0:4], stt[:, 0:4], stt[:, 4:8],
                                    op=mybir.AluOpType.add)
            st1 = stt[:, 0:4]    # [S_mt0, S_mt1, Q_mt0, Q_mt1]; Q pre-div N1
            bn1_in = dram.tile([128, 4], F32)
            bn1_out = dram.tile([NCORES, 128, 4], F32, addr_space="Shared")
            nc.scalar.dma_start(bn1_in[:], st1)
            nc.gpsimd.collective_compute(
                "AllGather", mybir.AluOpType.bypass,
                replica_groups=[list(range(NCORES))],
                ins=[bn1_in.opt()], outs=[bn1_out.opt()])
            # Sqrt table preload during the AllGather wait; the stt read
            # keeps it ordered after the stats Squares, scale=0 + eps bias
            # keeps the argument in sqrt's valid range
            nc.scalar.activation(dummy[:, 4:5], stt[0:1, 0:1],
                                 mybir.ActivationFunctionType.Sqrt,
                                 scale=0.0, bias=epst[0:1, 0:1])
            # half1's im2col casts drain during the AllGather
            _emit_casts(nc, cast_work.pop(), h1sb)
            stg = sp.tile([128, NCORES * 4], F32)
            nc.scalar.dma_start(
                stg[:].rearrange("p (r t) -> p r t", r=NCORES),
                bass.AP(bn1_out.tensor, 0, [[4, 128], [128 * 4, NCORES], [1, 4]]))
            stgr = stg[:].rearrange("p (r t) -> p r t", r=NCORES)
            for halfn in (4, 2, 1):
                nc.vector.tensor_tensor(
                    stgr[:, 0:halfn], stgr[:, 0:halfn],
                    stgr[:, halfn:2 * halfn], op=mybir.AluOpType.add)

            # BN1 coeffs, scale-first so conv2's weight scaling can start
            # before the shift path finishes. BN1 is folded into conv2:
            # the per-input-channel scale goes into w2 (h1sb's zero padding
            # must stay zero), the shift's contribution is a per-boundary-
            # class constant from 8 tiny matmuls against host-precomputed
            # class row sums.
            t1 = sp.tile([128, 8], F32, name="bn1t")
            mean1, var1, sd1, tn1 = (t1[:, i * 2:(i + 1) * 2] for i in range(4))
            nc.vector.tensor_scalar(mean1, stg[:, 0:2], 1.0 / N1, None,
                                    op0=mybir.AluOpType.mult)
            nc.vector.tensor_tensor(var1, mean1, mean1, op=mybir.AluOpType.mult)
            nc.vector.tensor_tensor(var1, stg[:, 2:4], var1,
                                    op=mybir.AluOpType.subtract)
            nc.scalar.activation(sd1, var1, mybir.ActivationFunctionType.Sqrt,
                                 bias=epst[:, 0:1])
            co1 = sp.tile([128, 6], F32, name="bn1c")
            scale1, shift1, r1 = co1[:, 0:2], co1[:, 2:4], co1[:, 4:6]
            nc.vector.reciprocal(r1, sd1)
            nc.vector.tensor_tensor(scale1, bsb[:, BC_BN1G:BC_BN1G + 2], r1,
                                    op=mybir.AluOpType.mult)
            for cb2 in range(2):
                nc.vector.tensor_scalar(
                    w2sb[:, cb2 * 1152:(cb2 + 1) * 1152],
                    w2sb[:, cb2 * 1152:(cb2 + 1) * 1152],
                    scale1[:, cb2:cb2 + 1], None, op0=mybir.AluOpType.mult)
            nc.vector.tensor_tensor(tn1, mean1, scale1, op=mybir.AluOpType.mult)
            nc.vector.tensor_tensor(shift1, bsb[:, BC_BN1B:BC_BN1B + 2], tn1,
                                    op=mybir.AluOpType.subtract)
            shift1b = sp.tile([128, 2], BF16)
            nc.vector.tensor_copy(shift1b[:], shift1)
            Tm = cps.tile([128, 4], F32, tag="c1ps")
            for cls in range(4):
                for cb2 in range(2):
                    nc.tensor.matmul(Tm[:, cls:cls + 1],
                                     w2s_sb[:, (cls * 2 + cb2) * 128:
                                            (cls * 2 + cb2 + 1) * 128],
                                     shift1b[:, cb2:cb2 + 1],
                                     start=(cb2 == 0), stop=(cb2 == 1),
                                     skip_group_check=True)
            TmS = sp.tile([128, 4], F32)
            nc.vector.tensor_copy(TmS[:], Tm[:])
            # broadcast the class constants into a full [128,200] map while
            # conv2 runs, so the post-conv2 merge is a single add
            Tmap = sp.tile([128, BL * P2], F32)
            Tv = Tmap[:].rearrange("p (i j n) -> p i j n", i=5, j=5, n=BL)
            for cls, sl in ((3, (slice(0, 1), slice(0, 1))),
                            (2, (slice(0, 1), slice(1, 5))),
                            (1, (slice(1, 5), slice(0, 1))),
                            (0, (slice(1, 5), slice(1, 5)))):
                ni = sl[0].stop - sl[0].start
                nj = (sl[1].stop - sl[1].start) * BL
                src = TmS[:, cls:cls + 1, None].to_broadcast([128, ni, nj])
                nc.vector.tensor_copy(
                    Tv[:, sl[0], sl[1]].rearrange("p a b c -> p a (b c)"), src)

            # ---------------- conv2 (one 18-matmul chain) ---------------
            c2p = cps.tile([128, P2 * BL], F32, name="c2p", tag="c1ps")
            for idx in range(18):
                cb2, kidx = idx // 9, idx % 9
                lhsT = w2sb[:, idx * 128:(idx + 1) * 128]
                nc.tensor.matmul(c2p[:], lhsT,
                                 h1sb[cb2][:, kidx * BLK:(kidx + 1) * BLK],
                                 start=(idx == 0), stop=(idx == 17))
            # psum -> sbuf bf16, adding the BN1-shift map in one pass
            c2sb = sp.tile([128, BL * P2], BF16)
            nc.vector.tensor_tensor(c2sb[:], Tmap[:], c2p[:],
                                    op=mybir.AluOpType.add)

            # ---------------- BN2 stats + AllGather (before matvec) -----
            st2l = sp.tile([128, 2], F32)
            nc.vector.reduce_sum(st2l[:, 0:1], c2sb[:], axis=mybir.AxisListType.X)
            sc2 = sp.tile([128, BL * P2], F32)
            nc.scalar.activation(sc2[:], c2sb[:],
                                 mybir.ActivationFunctionType.Square,
                                 scale=ISQ2, accum_out=st2l[:, 1:2])
            bn2_in = dram.tile([128, 2], F32)
            bn2_out = dram.tile([NCORES, 128, 2], F32, addr_space="Shared")
            nc.scalar.dma_start(bn2_in[:], st2l[:])
            nc.gpsimd.collective_compute(
                "AllGather", mybir.AluOpType.bypass,
                replica_groups=[list(range(NCORES))],
                ins=[bn2_in.opt()], outs=[bn2_out.opt()])

            # during the AllGather: weff partial matvec + Sigmoid table +
            # the pieces of the finish that don't need stats
            mvt = sp.tile([128, P2 * BL], F32)
            wb = weff[:, 0:25, None].to_broadcast([128, 25, BL])
            nc.vector.tensor_tensor(
                mvt[:].rearrange("p (i n) -> p i n", i=P2),
                c2sb[:].rearrange("p (i n) -> p i n", i=P2), wb,
                op=mybir.AluOpType.mult)
            Av = sp.tile([128, BL], F32)
            nc.vector.reduce_sum(Av[:], mvt[:].rearrange("p (i n) -> p n i", i=P2),
                                 axis=mybir.AxisListType.X)
            Avb = sp.tile([128, BL], BF16)
            nc.vector.tensor_copy(Avb[:], Av[:])
            nc.scalar.activation(dummy[:, 4:5], st2l[0:1, 0:1],
                                 mybir.ActivationFunctionType.Sigmoid)
            ones = wp.tile([128, BL], BF16)
            nc.gpsimd.memset(ones[:], 1.0)

            stg2 = sp.tile([128, NCORES * 2], F32)
            nc.scalar.dma_start(
                stg2[:].rearrange("p (r t) -> p r t", r=NCORES),
                bass.AP(bn2_out.tensor, 0, [[2, 128], [128 * 2, NCORES], [1, 2]]))
            stg2r = stg2[:].rearrange("p (r t) -> p r t", r=NCORES)
            for halfn in (4, 2, 1):
                nc.vector.tensor_tensor(
                    stg2r[:, 0:halfn], stg2r[:, 0:halfn],
                    stg2r[:, halfn:2 * halfn], op=mybir.AluOpType.add)

            # ---------------- BN2 coeffs + collapsed MLP finish ---------
            # z[n] = sum_c s2[c]*A[c,n] + sum_c shift2[c]*rowsum_weff[c];
            # scale-first so the first matmul issues while the shift path
            # is still on DVE
            t2 = sp.tile([128, 4], F32, name="bn2t")
            mean2, var2, sd2, tn2 = (t2[:, i:i + 1] for i in range(4))
            nc.vector.tensor_scalar(mean2, stg2[:, 0:1], 1.0 / N2, None,
                                    op0=mybir.AluOpType.mult)
            nc.vector.tensor_tensor(var2, mean2, mean2, op=mybir.AluOpType.mult)
            nc.vector.tensor_tensor(var2, stg2[:, 1:2], var2,
                                    op=mybir.AluOpType.subtract)
            nc.scalar.activation(sd2, var2, mybir.ActivationFunctionType.Sqrt,
                                 bias=epst[:, 0:1])
            co2 = sp.tile([128, 3], F32, name="bn2c")
            scale2, shift2, r2 = co2[:, 0:1], co2[:, 1:2], co2[:, 2:3]
            nc.vector.reciprocal(r2, sd2)
            nc.vector.tensor_tensor(scale2, bsb[:, BC_BN2G:BC_BN2G + 1], r2,
                                    op=mybir.AluOpType.mult)
            s2b = sp.tile([128, 1], BF16)
            nc.vector.tensor_copy(s2b[:], scale2)
            zps = cps.tile([1, BL], F32, tag="c1ps")
            nc.tensor.matmul(zps[:], s2b[:], Avb[:], start=True, stop=False)
            nc.vector.tensor_tensor(tn2, mean2, scale2, op=mybir.AluOpType.mult)
            nc.vector.tensor_tensor(shift2, bsb[:, BC_BN2B:BC_BN2B + 1], tn2,
                                    op=mybir.AluOpType.subtract)
            vsh = wp.tile([128, 1], BF16)
            nc.vector.tensor_tensor(vsh[:], shift2, weff[:, 25:26],
                                    op=mybir.AluOpType.mult)
            nc.tensor.matmul(zps[:], vsh[:], ones[:], start=False, stop=True)
            osb = sp.tile([1, BL], F32)
            nc.scalar.activation(osb[:], zps[:],
                                 mybir.ActivationFunctionType.Sigmoid,
                                 bias=bsb[0:1, BC_BEFF:BC_BEFF + 1])
            nc.sync.dma_start(bass.AP(out, 0, [[1, 1], [1, BL]]), osb[:])

    nc.compile()
    return nc


# ----------------------------------------------------------------------------
# host-side input prep
# ----------------------------------------------------------------------------

def _prep_inputs(inputs):
    import ml_dtypes
    f = np.float32
    bf = ml_dtypes.bfloat16
    x = np.asarray(inputs["x"], dtype=f)

    # conv1 patches: [n64, cb4, c128, i14, ki3, j14, kj3]
    xpad = np.zeros((B, 512, 42, 42), dtype=bf)
    xpad[:, :, 1:41, 1:41] = x.astype(bf)
    # -> [k9, cb4, c128, n64, pos196]
    xv = (xpad.reshape(B, 4, 128, 14, 3, 14, 3)
          .transpose(4, 6, 1, 2, 0, 3, 5)        # ki,kj,cb,c,n,i,j
          .reshape(9, 4, 128, B, P1))

    w1 = np.asarray(inputs["conv1_w"], dtype=f)          # [256, 512, 3, 3]
    # [128c, 9k, 4cb, 256m]
    w1p = np.ascontiguousarray(
        w1.reshape(256, 4, 128, 9).transpose(2, 3, 1, 0)).reshape(
            128, 9, 1024).astype(bf)
    w2 = np.asarray(inputs["conv2_w"], dtype=f)          # [128, 256, 3, 3]
    w2p = np.ascontiguousarray(
        w2.reshape(128, 2, 128, 9).transpose(2, 1, 3, 0)).reshape(
            128, 18, 128).astype(bf)
    # boundary-class row sums for the BN1-shift term:
    # class c = a*2+b, a=(i==0) -> ki>=1 only, b=(j==0) -> kj>=1 only
    w2r = w2.reshape(128, 2, 128, 3, 3)                  # c2, cb, c1w, ki, kj
    w2sp = np.zeros((128, 8 * 128), dtype=f)
    for cls in range(4):
        a, b = cls // 2, cls % 2
        kis = slice(1, 3) if a else slice(0, 3)
        kjs = slice(1, 3) if b else slice(0, 3)
        s = w2r[:, :, :, kis, kjs].sum(axis=(3, 4))      # c2, cb, c1w
        for cb in range(2):
            w2sp[:, (cls * 2 + cb) * 128:(cls * 2 + cb + 1) * 128] = s[:, cb].T
    w2sp = w2sp.astype(bf)

    # compose the 12 affine layers (no nonlinearities) into [3200] + scalar
    M = np.asarray(inputs["w14"], dtype=np.float64)      # [1, 2]
    beff = np.asarray(inputs["b14"], dtype=np.float64).copy()  # [1]
    for li in range(13, 2, -1):                          # w13 .. w3
        beff += M @ np.asarray(inputs[f"b{li}"], dtype=np.float64)
        M = M @ np.asarray(inputs[f"w{li}"], dtype=np.float64)
    weff = M.reshape(3200).astype(f)                     # order f = c*25 + ij
    w2d = weff.reshape(128, 25)
    weffp = np.zeros((128, 26), dtype=f)
    weffp[:, 0:25] = w2d
    weffp[:, 25] = w2d.sum(axis=1)
    beff_f = float(beff[0])

    bp = np.zeros((128, 7), dtype=f)
    bp[:, 0:2] = np.asarray(inputs["bn1_g"], dtype=f).reshape(2, 128).T
    bp[:, 2:4] = np.asarray(inputs["bn1_b"], dtype=f).reshape(2, 128).T
    bp[:, 4] = np.asarray(inputs["bn2_g"], dtype=f)
    bp[:, 5] = np.asarray(inputs["bn2_b"], dtype=f)
    bp[0, 6] = beff_f

    in_maps = []
    for r in range(NCORES):
        # [9k, 4cb, 128, 8n, 196] -> [9k, 2half, 128, 4cb, 2ptin, 2n, 196]
        xr = np.ascontiguousarray(
            xv[:, :, :, r * BL:(r + 1) * BL]
            .reshape(9, 4, 128, 2, 2, 2, P1)     # k, cb, c, half, ptin, n2, pos
            .transpose(0, 3, 2, 1, 4, 5, 6)
        ).reshape(9, 2, 128, 4 * 2 * PTW)
        in_maps.append({
            "xprep": xr, "w1p": w1p, "w2p": w2p, "w2sp": w2sp,
            "weffp": weffp, "bprep": bp,
        })
    return in_maps


def kernel(**inputs):
    if "nc" not in _CACHE:
        _CACHE["nc"] = _build()
    nc = _CACHE["nc"]
    in_maps = _prep_inputs(inputs)
    trace = bool(int(os.environ.get("KERNEL_TRACE", "0")))
    if trace:
        try:
            import ntff_shim
            ntff_shim.install()
        except ImportError:
            trace = False
    res = run_bass_kernel_spmd(nc, in_maps, core_ids=list(range(NCORES)),
                               trace=trace)
    _CACHE["last_result"] = res
    return np.concatenate([res.results[r]["out"] for r in range(NCORES)], axis=0)
